# revision 21
# baseline (speedup 1.0000x reference)
"""Trainium2 Bass kernel for nn_AttentionBlock (GroupNorm -> QKV 1x1 -> softmax
attention over 4096 tokens -> proj + residual).

Sharding: pure data-parallel over batch B=8 across the 8 NeuronCores (one
batch element per core); attention is per-batch-element so no collectives.

Per-core layout (C=512 channels, N=4096 tokens), all matmuls fp8e4 DoubleRow:
  - x loaded [channel-part, token] as 4 chunks of [128, 4096], DMA'd per
    512-col slice FIRST (before weights) so GroupNorm stats pipeline behind
    the DMA instead of serializing after it
  - GroupNorm fp32 (bn_stats per slice; group reduce/broadcast via tiny
    matmuls); h stored fp8 in DoubleRow pair layout [128, 2, 4096]
  - weights pre-scaled by powers of 2 so every fp8 tensor sits in e4m3's
    sweet spot; compensation folded into activation scales:
      wq,wk *= C^-0.25 * 32 (q,k stored as q*C^-0.25*4 -> logits' = 16*logit,
      EXP uses scale=1/16); wv *= 8 (v epilogue scale 1/8); wp *= 8
  - logits computed transposed: E^T[m, n] = sum_o k[o,m] q[o,n]; softmax via
    exp(logit - 2.5) in fp8 (shift cancels in normalization); denominator
    via a 0.25-valued ones-matmul so reciprocal gives 4/S directly
  - h_attn_unnorm[o, n] = sum_m vT[m, o] expT[m, n]; normalized by 4/S on
    the Vector engine into fp8 pairs (keeps Scalar engine pure-EXP during
    attention -> no ACT table thrash), then proj as fp8 DR one n-block
    behind, its 4 output groups interleaved into the next block's m-loop
  - final: out = pp/32 + pb + x (residual re-DMA'd, overlapped)

Self-contained: hardcodes shapes; builds + compiles the Bass graph once and
caches a persistent jitted shard_map executor over the 8 axon NeuronCores.
"""

import os
import sys

sys.path.insert(0, "/opt/trn_rl_repo")
os.environ.setdefault("MYCRO_LOCAL_CACHE", "1")

import numpy as np
import ml_dtypes

BF16 = ml_dtypes.bfloat16
FP8 = ml_dtypes.float8_e4m3

# Problem constants (hardcoded; kernel.py must not read spec/reference files)
B, C, H, W = 8, 512, 64, 64
N = H * W            # 4096 tokens
P = 128              # partitions
NCH = C // P         # 4 channel chunks
NOP = NCH // 2       # 2 channel-chunk pairs (DoubleRow)
BW = 512             # n-block width (= PSUM bank in fp32)
NB = N // BW         # 8 n-blocks
MT = N // P          # 32 m-tiles
MPAIR = MT // 2      # 16 m-tile pairs (DoubleRow)
G = 32               # groups
GS = C // G          # 16 channels per group
GPC = P // GS        # 8 groups per 128-channel chunk
NSG = 8              # GN stat slices per chunk (512 cols each)
EPS = 1e-6
EXP_SHIFT = 2.5      # exp(logit - shift); cancels in softmax normalization
NCORES = 8

# fp8 scale management (powers of 2; see module docstring)
QK_WBOOST = 32.0     # wq,wk stored * C^-0.25 * 32
QK_DSCALE = 0.125    # q epilogue: q' = psum/8 + qb'  (q' = q * C^-0.25 * 4)
EXP_SCALE = 0.0625   # logits' = 16 * logits
V_WBOOST = 8.0       # wv stored * 8
V_DSCALE = 0.125     # v epilogue: v' = psum/8 + vb  (v' = v)
P_WBOOST = 8.0       # wp stored * 8
ONES_VAL = 0.25      # ps_s = S/4 -> r = 4/S; hu = h_unnorm * 4/S
P_DSCALE = 1.0 / 32  # out = psum/32 + pb + x  (psum = 8*wp . 4*h_norm)

_EXEC = None


def _build_nc():
    import concourse.bacc as bacc
    import concourse.tile as tile
    from concourse import mybir

    f32 = mybir.dt.float32
    fp8 = mybir.dt.float8e4
    Alu = mybir.AluOpType
    Act = mybir.ActivationFunctionType
    DR = mybir.MatmulPerfMode.DoubleRow

    nc = bacc.Bacc(
        "TRN2", target_bir_lowering=False, debug=False, num_devices=NCORES
    )

    def din(name, shape, dt=f32):
        return nc.declare_dram_parameter(name, list(shape), dt, isOutput=False)

    x_d = din("x", [C, N])
    wq_d = din("wq", [NOP, P, 2, C], fp8)   # pair layout, pre-scaled
    wk_d = din("wk", [NOP, P, 2, C], fp8)
    wv_d = din("wv", [NOP, P, 2, C], fp8)
    wp_d = din("wp", [NOP, P, 2, C], fp8)
    # [gamma, beta, qb, kb, pb] columns; pb has proj_w @ v_b folded in
    # (softmax sums to 1) -> one DMA for all per-channel vectors
    vecs_d = din("vecs", [C, 5])
    selsum_d = din("selsum", [P, GPC])
    selbc_d = din("selbc", [GPC, P])
    ones8_d = din("ones8", [P, 2 * P], fp8)   # DoubleRow 0.25s [P, 2, P]
    out_d = nc.declare_dram_parameter("out", [C, N], f32, isOutput=True)

    with tile.TileContext(nc) as tc:
        with (
            tc.tile_pool(name="consts", bufs=1) as consts,
            tc.tile_pool(name="qksb", bufs=1) as qkp,
            tc.tile_pool(name="vtsb", bufs=1) as vtp,
            tc.tile_pool(name="xsb", bufs=1) as xp,
        ):
            # ---- small consts first (GroupNorm needs them immediately);
            # batched into 4 DMAs (each dma_start costs ~650ns of sequencer
            # descriptor-gen, so fewer+bigger wins) ----
            vecs_sb = consts.tile([P, NCH, 5], f32, tag="vecs")
            nc.sync.dma_start(
                out=vecs_sb,
                in_=vecs_d[:, :].rearrange("(c p) v -> p c v", p=P),
            )
            gamma_sb = [vecs_sb[:, cc, 0:1] for cc in range(NCH)]
            beta_sb = [vecs_sb[:, cc, 1:2] for cc in range(NCH)]
            qb_sb = [vecs_sb[:, cc, 2:3] for cc in range(NCH)]
            kb_sb = [vecs_sb[:, cc, 3:4] for cc in range(NCH)]
            pb_sb = [vecs_sb[:, cc, 4:5] for cc in range(NCH)]
            selsum_sb = consts.tile([P, GPC], f32, tag="selsum")
            nc.sync.dma_start(out=selsum_sb, in_=selsum_d[:, :])
            selbc_sb = consts.tile([P, P], f32, tag="selbc")
            nc.sync.dma_start(out=selbc_sb[0:GPC, :], in_=selbc_d[:, :])
            ones8_sb = consts.tile([P, 2, P], fp8, tag="ones8")
            nc.sync.dma_start(
                out=ones8_sb,
                in_=ones8_d[:, :].rearrange("p (j q) -> p j q", j=2),
            )
            eps_sb = consts.tile([P, 1], f32, tag="eps")
            nc.vector.memset(eps_sb, EPS)
            negc_sb = consts.tile([P, 1], f32, tag="negc")
            nc.vector.memset(negc_sb, -EXP_SHIFT)

            # ---- x DMAs next (GN critical path); whole-chunk transfers
            # spread across all DMA queues internally ----
            x_sb = []
            for cc in range(NCH):
                xt = xp.tile([P, N], f32, tag=f"x{cc}", name=f"x{cc}")
                nc.sync.dma_start(out=xt, in_=x_d[cc * P : (cc + 1) * P, :])
                x_sb.append(xt)

            # ---- weights on the gpsimd queue (concurrent with x; only
            # 1 MiB so they land early without delaying the x stream) ----
            def wpairs(d, tagp):
                ts = []
                for p in range(NOP):
                    t = consts.tile([P, 2, C], fp8, tag=f"{tagp}{p}",
                                    name=f"{tagp}{p}")
                    nc.gpsimd.dma_start(out=t, in_=d[p, :, :, :])
                    ts.append(t)
                return ts

            wq_sb = wpairs(wq_d, "wq")
            wk_sb = wpairs(wk_d, "wk")
            wv_sb = wpairs(wv_d, "wv")
            wp_sb = wpairs(wp_d, "wp")

            # q/k in DoubleRow pair layout: [P, 2, N], dim1 = pair member j,
            # channel chunk oc = 2*op + j; vT pairs [token-part, 2, C]
            q_sb = [qkp.tile([P, 2, N], fp8, tag=f"q{op}", name=f"q{op}")
                    for op in range(NOP)]
            k_sb = [qkp.tile([P, 2, N], fp8, tag=f"k{op}", name=f"k{op}")
                    for op in range(NOP)]
            vt_sb = [vtp.tile([P, 2, C], fp8, tag=f"vt{t}", name=f"vt{t}")
                     for t in range(MPAIR)]

            with tc.tile_pool(name="hsb", bufs=1) as hp:
                # h in fp8 pair layout for DR QKV matmuls
                h_sb = [hp.tile([P, 2, N], fp8, tag=f"h{p}", name=f"h{p}")
                        for p in range(NOP)]
                # ---- GroupNorm (per 128-channel chunk; groups don't cross) --
                with (
                    tc.tile_pool(name="gn", bufs=2) as gn,
                    tc.tile_pool(name="gnps", bufs=2, space="PSUM") as gnps,
                ):
                    for cc in range(NCH):
                        xt = x_sb[cc]
                        stats = gn.tile([P, NSG, 6], f32, tag="stats")
                        sw = N // NSG
                        for sg in range(NSG):
                            nc.vector.bn_stats(
                                out=stats[:, sg, :],
                                in_=xt[:, sg * sw : (sg + 1) * sw],
                            )
                        mv = gn.tile([P, 2], f32, tag="mv")
                        nc.vector.bn_aggr(out=mv, in_=stats)
                        # rhs2 = [mean_c, E[x^2]_c]
                        rhs2 = gn.tile([P, 2], f32, tag="rhs2")
                        nc.vector.tensor_copy(out=rhs2[:, 0:1], in_=mv[:, 0:1])
                        nc.vector.scalar_tensor_tensor(
                            out=rhs2[:, 1:2], in0=mv[:, 0:1], scalar=mv[:, 0:1],
                            in1=mv[:, 1:2], op0=Alu.mult, op1=Alu.add,
                        )
                        # group sums over the 16 channels of each group
                        g_ps = gnps.tile([P, 2], f32, tag="g_ps")
                        nc.tensor.matmul(
                            out=g_ps[0:GPC, :], lhsT=selsum_sb, rhs=rhs2,
                            start=True, stop=True,
                        )
                        gs_t = gn.tile([P, 2], f32, tag="gs")
                        nc.vector.tensor_scalar(
                            out=gs_t[0:GPC, :], in0=g_ps[0:GPC, :],
                            scalar1=1.0 / GS, scalar2=None, op0=Alu.mult,
                        )
                        mean2 = gn.tile([P, 1], f32, tag="mean2")
                        nc.vector.tensor_mul(mean2[0:GPC], gs_t[0:GPC, 0:1],
                                             gs_t[0:GPC, 0:1])
                        var = gn.tile([P, 1], f32, tag="var")
                        nc.vector.tensor_sub(var[0:GPC], gs_t[0:GPC, 1:2],
                                             mean2[0:GPC])
                        sq = gn.tile([P, 1], f32, tag="sq")
                        nc.scalar.activation(
                            out=sq[0:GPC], in_=var[0:GPC], func=Act.Sqrt,
                            bias=eps_sb[0:GPC], scale=1.0,
                        )
                        gmr = gn.tile([P, 2], f32, tag="gmr")
                        nc.vector.tensor_copy(out=gmr[0:GPC, 0:1],
                                              in_=gs_t[0:GPC, 0:1])
                        nc.vector.reciprocal(out=gmr[0:GPC, 1:2], in_=sq[0:GPC])
                        # broadcast (mean_g, rstd_g) back to channels
                        bc_ps = gnps.tile([P, 2], f32, tag="bc_ps")
                        nc.tensor.matmul(
                            out=bc_ps, lhsT=selbc_sb[0:GPC, :], rhs=gmr[0:GPC, :],
                            start=True, stop=True,
                        )
                        a_t = gn.tile([P, 1], f32, tag="a")
                        nc.vector.tensor_mul(a_t, bc_ps[:, 1:2], gamma_sb[cc])
                        na_t = gn.tile([P, 1], f32, tag="na")
                        nc.vector.tensor_scalar_mul(na_t, a_t, -1.0)
                        b_t = gn.tile([P, 1], f32, tag="b")
                        nc.vector.scalar_tensor_tensor(
                            out=b_t, in0=bc_ps[:, 0:1], scalar=na_t,
                            in1=beta_sb[cc], op0=Alu.mult, op1=Alu.add,
                        )
                        # h = x*a + b -> fp8 pair slice; last chunk on DVE
                        # (shorter critical path into the QKV matmuls)
                        if cc < NCH - 1:
                            nc.scalar.activation(
                                out=h_sb[cc // 2][:, cc % 2, :], in_=xt,
                                func=Act.Identity, scale=a_t, bias=b_t,
                            )
                        else:
                            nc.vector.tensor_scalar(
                                out=h_sb[cc // 2][:, cc % 2, :], in0=xt,
                                scalar1=a_t, scalar2=b_t,
                                op0=Alu.mult, op1=Alu.add,
                            )

                # ---- QKV projections (all fp8 DoubleRow); PSUM->fp8
                # epilogues alternate ACT/DVE so neither engine gates PE ----
                with tc.tile_pool(name="qkvps", bufs=4, space="PSUM") as qkvps:
                    for w_sb, b_sb, dst in ((wq_sb, qb_sb, q_sb),
                                            (wk_sb, kb_sb, k_sb)):
                        for oc in range(NCH):
                            osl = slice(oc * P, (oc + 1) * P)
                            for nt in range(NB):
                                nsl = slice(nt * BW, (nt + 1) * BW)
                                pt = qkvps.tile([P, BW], f32, tag="qkv")
                                for p in range(NOP):
                                    nc.tensor.matmul(
                                        out=pt,
                                        lhsT=w_sb[p][:, 0:2, osl],
                                        rhs=h_sb[p][:, 0:2, nsl],
                                        start=(p == 0), stop=(p == NOP - 1),
                                        perf_mode=DR,
                                    )
                                if nt % 2 == 0:
                                    nc.scalar.activation(
                                        out=dst[oc // 2][:, oc % 2, nsl],
                                        in_=pt, func=Act.Identity,
                                        scale=QK_DSCALE, bias=b_sb[oc],
                                    )
                                else:
                                    nc.vector.tensor_scalar(
                                        out=dst[oc // 2][:, oc % 2, nsl],
                                        in0=pt, scalar1=QK_DSCALE,
                                        scalar2=b_sb[oc],
                                        op0=Alu.mult, op1=Alu.add,
                                    )
                    # vT[m, o] = sum_c h[c, m] wv[c, o]; v_b is folded into
                    # pb on the host (softmax weights sum to 1)
                    for mt in range(MT):
                        msl = slice(mt * P, (mt + 1) * P)
                        pt = qkvps.tile([P, BW], f32, tag="qkv")
                        for p in range(NOP):
                            nc.tensor.matmul(
                                out=pt, lhsT=h_sb[p][:, 0:2, msl],
                                rhs=wv_sb[p], start=(p == 0),
                                stop=(p == NOP - 1), perf_mode=DR,
                            )
                        if mt % 2 == 0:
                            nc.scalar.activation(
                                out=vt_sb[mt // 2][:, mt % 2, :], in_=pt,
                                func=Act.Identity, scale=V_DSCALE,
                            )
                        else:
                            nc.vector.tensor_scalar(
                                out=vt_sb[mt // 2][:, mt % 2, :], in0=pt,
                                scalar1=V_DSCALE, scalar2=None, op0=Alu.mult,
                            )

            # ---- attention (fp8 DR) + interleaved delayed proj + residual --
            with (
                tc.tile_pool(name="eps_ps", bufs=2, space="PSUM") as e_ps,
                tc.tile_pool(name="s_ps", bufs=1, space="PSUM") as s_ps,
                tc.tile_pool(name="h_ps", bufs=1, space="PSUM") as h_ps,
                tc.tile_pool(name="p_ps", bufs=1, space="PSUM") as p_ps,
                tc.tile_pool(name="expt", bufs=8) as expt,
                tc.tile_pool(name="sumt", bufs=3) as sumt,
                tc.tile_pool(name="hup", bufs=2) as hup,
                tc.tile_pool(name="epil", bufs=2) as epil,
            ):
                def emit_proj_group(nbp, hu, oc2, last=False):
                    nsl = slice(nbp * BW, (nbp + 1) * BW)
                    osl = slice(oc2 * P, (oc2 + 1) * P)
                    pp = p_ps.tile([P, BW], f32, tag="p", name="pp")
                    for p in range(NOP):
                        nc.tensor.matmul(
                            out=pp, lhsT=wp_sb[p][:, 0:2, osl], rhs=hu[p],
                            start=(p == 0), stop=(p == NOP - 1), perf_mode=DR,
                        )
                    xt = epil.tile([P, BW], f32, tag="xt", name="xt")
                    nc.sync.dma_start(out=xt, in_=x_d[osl, nsl])
                    # out = pp/32 + pb + x; the last block's scale+bias goes
                    # on ACT (idle after the final EXP) to shorten the tail
                    t1 = epil.tile([P, BW], f32, tag="t1", name="t1")
                    if last:
                        nc.scalar.activation(
                            out=t1, in_=pp, func=Act.Identity,
                            scale=P_DSCALE, bias=pb_sb[oc2],
                        )
                    else:
                        nc.vector.tensor_scalar(
                            out=t1, in0=pp, scalar1=P_DSCALE,
                            scalar2=pb_sb[oc2], op0=Alu.mult, op1=Alu.add,
                        )
                    ot = epil.tile([P, BW], f32, tag="ot", name="ot")
                    nc.vector.tensor_tensor(out=ot, in0=t1, in1=xt, op=Alu.add)
                    nc.sync.dma_start(out=out_d[osl, nsl], in_=ot)

                def emit_av(pr, et, ph):
                    """attn.v matmuls for one m-pair (consume et)."""
                    for oc in range(NCH):
                        nc.tensor.matmul(
                            out=ph[oc],
                            lhsT=vt_sb[pr][:, 0:2, oc * P : (oc + 1) * P],
                            rhs=et,
                            start=(pr == 0), stop=(pr == MPAIR - 1),
                            perf_mode=DR,
                        )

                def emit_ones(g, sg, ps_s):
                    nc.tensor.matmul(
                        out=ps_s, lhsT=ones8_sb, rhs=sg,
                        start=(g == 0), stop=(g == MPAIR // 2 - 1),
                        perf_mode=DR,
                    )

                pending = None
                for nb in range(NB):
                    nsl = slice(nb * BW, (nb + 1) * BW)
                    ps_s = s_ps.tile([P, BW], f32, tag="s", name="ps_s")
                    ph = [h_ps.tile([P, BW], f32, tag=f"h{oc}", name=f"hps{oc}")
                          for oc in range(NCH)]
                    # software pipeline: AV runs one m-pair behind the E
                    # matmuls so PE never waits on the EXP latency; the
                    # softmax denominator sums groups of 4 exp tiles on the
                    # (otherwise idle) GpSimd engine so only one ones-matmul
                    # per group hits the PE
                    ets, sgs = [], []
                    for pr in range(MPAIR):
                        et = expt.tile([P, 2, BW], fp8, tag="et", name="et")
                        for j in range(2):
                            mt = 2 * pr + j
                            msl = slice(mt * P, (mt + 1) * P)
                            pe = e_ps.tile([P, BW], f32, tag="e", name="pe")
                            for op in range(NOP):
                                nc.tensor.matmul(
                                    out=pe, lhsT=k_sb[op][:, 0:2, msl],
                                    rhs=q_sb[op][:, 0:2, nsl],
                                    start=(op == 0), stop=(op == NOP - 1),
                                    perf_mode=DR,
                                )
                            nc.scalar.activation(
                                out=et[:, j, :], in_=pe, func=Act.Exp,
                                bias=negc_sb, scale=EXP_SCALE,
                            )
                        ets.append(et)
                        if pr % 2 == 1:
                            # elementwise pair-sum (values <~50 << fp8 max)
                            sg = sumt.tile([P, 2, BW], fp8, tag="sg",
                                           name="sg")
                            nc.gpsimd.tensor_tensor(
                                out=sg, in0=ets[pr - 1], in1=ets[pr],
                                op=Alu.add,
                            )
                            sgs.append(sg)
                        if pr > 0:
                            emit_av(pr - 1, ets[pr - 1], ph)
                            # spread prev block's proj through the m-loop
                            if pending is not None and pr % 4 == 0:
                                emit_proj_group(pending[0], pending[1],
                                                pr // 4 - 1)
                            # ones-matmul for pair-group g two m-pairs after
                            # its gpsimd sum was issued (hides the ~2.4us add)
                            if pr % 2 == 1 and pr >= 3:
                                emit_ones(pr // 2 - 1, sgs[pr // 2 - 1], ps_s)
                    emit_av(MPAIR - 1, ets[MPAIR - 1], ph)
                    if pending is not None:
                        emit_proj_group(pending[0], pending[1], NCH - 1)
                    emit_ones(MPAIR // 2 - 1, sgs[MPAIR // 2 - 1], ps_s)
                    # r = 4/S (ones are 0.25s); normalize into fp8 pairs on
                    # DVE (keeps ACT pure-EXP)
                    r_t = epil.tile([P, BW], f32, tag="r", name="r_t")
                    nc.vector.reciprocal_approx_fast(out=r_t, in_=ps_s)
                    hu = [hup.tile([P, 2, BW], fp8, tag=f"hu{p}",
                                   name=f"hu{p}") for p in range(NOP)]
                    for oc in range(NCH):
                        nc.vector.tensor_tensor(
                            out=hu[oc // 2][:, oc % 2, :], in0=ph[oc],
                            in1=r_t, op=Alu.mult,
                        )
                    pending = (nb, hu)
                for oc2 in range(NCH):
                    emit_proj_group(pending[0], pending[1], oc2, last=True)

    nc.compile()
    return nc


def _build_exec():
    import jax
    from jax.experimental.shard_map import shard_map
    from jax.sharding import Mesh, PartitionSpec

    from concourse import bass2jax, mybir

    nc = _build_nc()
    bass2jax.install_neuronx_cc_hook()

    partition_name = (
        nc.partition_id_tensor.name if nc.partition_id_tensor else None
    )
    in_names, out_names, out_avals = [], [], []
    for alloc in nc.m.functions[0].allocations:
        if not isinstance(alloc, mybir.MemoryLocationSet):
            continue
        name = alloc.memorylocations[0].name
        if alloc.kind == "ExternalInput":
            if name != partition_name:
                in_names.append(name)
        elif alloc.kind == "ExternalOutput":
            out_names.append(name)
            out_avals.append(
                jax.core.ShapedArray(
                    tuple(alloc.tensor_shape), mybir.dt.np(alloc.dtype)
                )
            )
    n_params = len(in_names)
    all_in = tuple(in_names + out_names)
    if partition_name is not None:
        all_in = all_in + (partition_name,)
    donate = tuple(range(n_params, n_params + len(out_names)))

    def _body(*args):
        operands = list(args)
        if partition_name is not None:
            operands.append(bass2jax.partition_id_tensor())
        outs = bass2jax._bass_exec_p.bind(
            *operands,
            out_avals=tuple(out_avals),
            in_names=all_in,
            out_names=tuple(out_names),
            lowering_input_output_aliases=(),
            sim_require_finite=True,
            sim_require_nnan=True,
            nc=nc,
        )
        return tuple(outs)

    devices = jax.devices()[:NCORES]
    mesh = Mesh(np.asarray(devices), ("core",))
    in_specs = (PartitionSpec("core"),) * (n_params + len(out_names))
    out_specs = (PartitionSpec("core"),) * len(out_names)
    sharded = jax.jit(
        shard_map(
            _body, mesh=mesh, in_specs=in_specs, out_specs=out_specs,
            check_rep=False,
        ),
        donate_argnums=donate,
        keep_unused=True,
    )
    return sharded, in_names, out_names, out_avals, nc


def _get_exec():
    global _EXEC
    if _EXEC is None:
        _EXEC = _build_exec()
    return _EXEC


def _selsum():
    s = np.zeros((P, GPC), np.float32)
    s[np.arange(P), np.arange(P) // GS] = 1.0
    return s


def _pair_fp8(w, boost):
    """[C_out, C_in] torch-style weight -> fp8 DR pair layout [NOP,P,2,C]."""
    wt = np.ascontiguousarray(np.asarray(w, np.float32).T) * boost
    return np.ascontiguousarray(
        wt.reshape(NOP, 2, P, C).transpose(0, 2, 1, 3)
    ).astype(FP8)


def make_concat_inputs(inputs):
    """Host-side prep: per-core shards concatenated on axis 0 (shard_map)."""
    x = np.asarray(inputs["x"], np.float32).reshape(B, C, N)
    sqs = np.float32(C ** -0.25)

    shared = {
        "wq": _pair_fp8(inputs["q_w"], sqs * QK_WBOOST),
        "wk": _pair_fp8(inputs["k_w"], sqs * QK_WBOOST),
        "wv": _pair_fp8(inputs["v_w"], V_WBOOST),
        "wp": _pair_fp8(inputs["proj_w"], P_WBOOST),
        # [gamma, beta, qb', kb', pb'] columns; pb' folds in proj_w @ v_b
        # (softmax weights sum to 1, so v_b is a constant proj-side shift)
        "vecs": np.ascontiguousarray(np.stack([
            np.asarray(inputs["gamma"], np.float32),
            np.asarray(inputs["beta"], np.float32),
            np.asarray(inputs["q_b"], np.float32)
            * (sqs * QK_WBOOST * QK_DSCALE),
            np.asarray(inputs["k_b"], np.float32)
            * (sqs * QK_WBOOST * QK_DSCALE),
            np.asarray(inputs["proj_b"], np.float32)
            + np.asarray(inputs["proj_w"], np.float32)
            @ np.asarray(inputs["v_b"], np.float32),
        ], axis=1)),
        "selsum": _selsum(),
        "selbc": np.ascontiguousarray(_selsum().T),
        "ones8": np.full((P, 2 * P), ONES_VAL, FP8),
    }
    per_core = [dict(shared, x=np.ascontiguousarray(x[c]))
                for c in range(NCORES)]

    sharded, in_names, out_names, out_avals, _ = _get_exec()
    concat_in = [
        np.concatenate([per_core[c][nm] for c in range(NCORES)], axis=0)
        for nm in in_names
    ]
    return concat_in, out_avals


def run_concat(concat_in, out_avals):
    sharded = _get_exec()[0]
    concat_zeros = [
        np.zeros((NCORES * av.shape[0], *av.shape[1:]), av.dtype)
        for av in out_avals
    ]
    outs = sharded(*concat_in, *concat_zeros)
    return outs


def kernel(**inputs):
    concat_in, out_avals = make_concat_inputs(inputs)
    outs = run_concat(concat_in, out_avals)
    o = np.asarray(outs[0]).reshape(NCORES, C, N)
    return np.ascontiguousarray(o.reshape(B, C, H, W), dtype=np.float32)


# revision 22
# speedup vs baseline: 1.0134x; 1.0134x over previous
"""Trainium2 Bass kernel for nn_AttentionBlock (GroupNorm -> QKV 1x1 -> softmax
attention over 4096 tokens -> proj + residual).

Sharding: pure data-parallel over batch B=8 across the 8 NeuronCores (one
batch element per core); attention is per-batch-element so no collectives.

Per-core layout (C=512 channels, N=4096 tokens), all matmuls fp8e4 DoubleRow:
  - x loaded [channel-part, token] as 4 chunks of [128, 4096], DMA'd per
    512-col slice FIRST (before weights) so GroupNorm stats pipeline behind
    the DMA instead of serializing after it
  - GroupNorm fp32 (bn_stats per slice; group reduce/broadcast via tiny
    matmuls); h stored fp8 in DoubleRow pair layout [128, 2, 4096]
  - weights pre-scaled by powers of 2 so every fp8 tensor sits in e4m3's
    sweet spot; compensation folded into activation scales:
      wq,wk *= C^-0.25 * 32 (q,k stored as q*C^-0.25*4 -> logits' = 16*logit,
      EXP uses scale=1/16); wv *= 8 (v epilogue scale 1/8); wp *= 8
  - logits computed transposed: E^T[m, n] = sum_o k[o,m] q[o,n]; softmax via
    exp(logit - 2.5) in fp8 (shift cancels in normalization); denominator
    via a 0.25-valued ones-matmul so reciprocal gives 4/S directly
  - h_attn_unnorm[o, n] = sum_m vT[m, o] expT[m, n]; normalized by 4/S on
    the Vector engine into fp8 pairs (keeps Scalar engine pure-EXP during
    attention -> no ACT table thrash), then proj as fp8 DR one n-block
    behind, its 4 output groups interleaved into the next block's m-loop
  - final: out = pp/32 + pb + x (residual re-DMA'd, overlapped)

Self-contained: hardcodes shapes; builds + compiles the Bass graph once and
caches a persistent jitted shard_map executor over the 8 axon NeuronCores.
"""

import os
import sys

sys.path.insert(0, "/opt/trn_rl_repo")
os.environ.setdefault("MYCRO_LOCAL_CACHE", "1")

import numpy as np
import ml_dtypes

BF16 = ml_dtypes.bfloat16
FP8 = ml_dtypes.float8_e4m3

# Problem constants (hardcoded; kernel.py must not read spec/reference files)
B, C, H, W = 8, 512, 64, 64
N = H * W            # 4096 tokens
P = 128              # partitions
NCH = C // P         # 4 channel chunks
NOP = NCH // 2       # 2 channel-chunk pairs (DoubleRow)
BW = 512             # n-block width (= PSUM bank in fp32)
NB = N // BW         # 8 n-blocks
MT = N // P          # 32 m-tiles
MPAIR = MT // 2      # 16 m-tile pairs (DoubleRow)
G = 32               # groups
GS = C // G          # 16 channels per group
GPC = P // GS        # 8 groups per 128-channel chunk
NSG = 8              # GN stat slices per chunk (512 cols each)
EPS = 1e-6
EXP_SHIFT = 2.5      # exp(logit - shift); cancels in softmax normalization
NCORES = 8

# fp8 scale management (powers of 2; see module docstring)
QK_WBOOST = 32.0     # wq,wk stored * C^-0.25 * 32
QK_DSCALE = 0.125    # q epilogue: q' = psum/8 + qb'  (q' = q * C^-0.25 * 4)
EXP_SCALE = 0.0625   # logits' = 16 * logits
V_WBOOST = 8.0       # wv stored * 8
V_DSCALE = 0.125     # v epilogue: v' = psum/8 + vb  (v' = v)
P_WBOOST = 8.0       # wp stored * 8
ONES_VAL = 0.25      # ps_s = S/4 -> r = 4/S; hu = h_unnorm * 4/S
P_DSCALE = 1.0 / 32  # out = psum/32 + pb + x  (psum = 8*wp . 4*h_norm)

_EXEC = None


def _build_nc():
    import concourse.bacc as bacc
    import concourse.tile as tile
    from concourse import mybir

    f32 = mybir.dt.float32
    fp8 = mybir.dt.float8e4
    Alu = mybir.AluOpType
    Act = mybir.ActivationFunctionType
    DR = mybir.MatmulPerfMode.DoubleRow

    nc = bacc.Bacc(
        "TRN2", target_bir_lowering=False, debug=False, num_devices=NCORES
    )

    def din(name, shape, dt=f32):
        return nc.declare_dram_parameter(name, list(shape), dt, isOutput=False)

    x_d = din("x", [C, N])
    wq_d = din("wq", [NOP, P, 2, C], fp8)   # pair layout, pre-scaled
    wk_d = din("wk", [NOP, P, 2, C], fp8)
    wv_d = din("wv", [NOP, P, 2, C], fp8)
    wp_d = din("wp", [NOP, P, 2, C], fp8)
    # [gamma, beta, qb, kb, pb] columns; pb has proj_w @ v_b folded in
    # (softmax sums to 1) -> one DMA for all per-channel vectors
    vecs_d = din("vecs", [C, 5])
    selsum_d = din("selsum", [P, GPC])
    selbc_d = din("selbc", [GPC, P])
    ones8_d = din("ones8", [P, 2 * P], fp8)   # DoubleRow 0.25s [P, 2, P]
    out_d = nc.declare_dram_parameter("out", [C, N], f32, isOutput=True)

    with tile.TileContext(nc) as tc:
        with (
            tc.tile_pool(name="consts", bufs=1) as consts,
            tc.tile_pool(name="qksb", bufs=1) as qkp,
            tc.tile_pool(name="vtsb", bufs=1) as vtp,
            tc.tile_pool(name="xsb", bufs=1) as xp,
        ):
            # ---- small consts first (GroupNorm needs them immediately);
            # batched into 4 DMAs (each dma_start costs ~650ns of sequencer
            # descriptor-gen, so fewer+bigger wins) ----
            vecs_sb = consts.tile([P, NCH, 5], f32, tag="vecs")
            nc.sync.dma_start(
                out=vecs_sb,
                in_=vecs_d[:, :].rearrange("(c p) v -> p c v", p=P),
            )
            gamma_sb = [vecs_sb[:, cc, 0:1] for cc in range(NCH)]
            beta_sb = [vecs_sb[:, cc, 1:2] for cc in range(NCH)]
            qb_sb = [vecs_sb[:, cc, 2:3] for cc in range(NCH)]
            kb_sb = [vecs_sb[:, cc, 3:4] for cc in range(NCH)]
            pb_sb = [vecs_sb[:, cc, 4:5] for cc in range(NCH)]
            selsum_sb = consts.tile([P, GPC], f32, tag="selsum")
            nc.sync.dma_start(out=selsum_sb, in_=selsum_d[:, :])
            selbc_sb = consts.tile([P, P], f32, tag="selbc")
            nc.sync.dma_start(out=selbc_sb[0:GPC, :], in_=selbc_d[:, :])
            ones8_sb = consts.tile([P, 2, P], fp8, tag="ones8")
            nc.sync.dma_start(
                out=ones8_sb,
                in_=ones8_d[:, :].rearrange("p (j q) -> p j q", j=2),
            )
            eps_sb = consts.tile([P, 1], f32, tag="eps")
            nc.vector.memset(eps_sb, EPS)
            negc_sb = consts.tile([P, 1], f32, tag="negc")
            nc.vector.memset(negc_sb, -EXP_SHIFT)

            # ---- x DMAs next (GN critical path); whole-chunk transfers
            # spread across all DMA queues internally ----
            x_sb = []
            for cc in range(NCH):
                xt = xp.tile([P, N], f32, tag=f"x{cc}", name=f"x{cc}")
                nc.sync.dma_start(out=xt, in_=x_d[cc * P : (cc + 1) * P, :])
                x_sb.append(xt)

            # ---- weights on the gpsimd queue (concurrent with x; only
            # 1 MiB so they land early without delaying the x stream) ----
            def wpairs(d, tagp):
                ts = []
                for p in range(NOP):
                    t = consts.tile([P, 2, C], fp8, tag=f"{tagp}{p}",
                                    name=f"{tagp}{p}")
                    nc.gpsimd.dma_start(out=t, in_=d[p, :, :, :])
                    ts.append(t)
                return ts

            wq_sb = wpairs(wq_d, "wq")
            wk_sb = wpairs(wk_d, "wk")
            wv_sb = wpairs(wv_d, "wv")
            wp_sb = wpairs(wp_d, "wp")

            # q/k in DoubleRow pair layout: [P, 2, N], dim1 = pair member j,
            # channel chunk oc = 2*op + j; vT pairs [token-part, 2, C]
            q_sb = [qkp.tile([P, 2, N], fp8, tag=f"q{op}", name=f"q{op}")
                    for op in range(NOP)]
            k_sb = [qkp.tile([P, 2, N], fp8, tag=f"k{op}", name=f"k{op}")
                    for op in range(NOP)]
            vt_sb = [vtp.tile([P, 2, C], fp8, tag=f"vt{t}", name=f"vt{t}")
                     for t in range(MPAIR)]

            with tc.tile_pool(name="hsb", bufs=1) as hp:
                # h in fp8 pair layout for DR QKV matmuls
                h_sb = [hp.tile([P, 2, N], fp8, tag=f"h{p}", name=f"h{p}")
                        for p in range(NOP)]
                # ---- GroupNorm (per 128-channel chunk; groups don't cross) --
                with (
                    tc.tile_pool(name="gn", bufs=2) as gn,
                    tc.tile_pool(name="gnps", bufs=2, space="PSUM") as gnps,
                ):
                    for cc in range(NCH):
                        xt = x_sb[cc]
                        stats = gn.tile([P, NSG, 6], f32, tag="stats")
                        sw = N // NSG
                        for sg in range(NSG):
                            nc.vector.bn_stats(
                                out=stats[:, sg, :],
                                in_=xt[:, sg * sw : (sg + 1) * sw],
                            )
                        mv = gn.tile([P, 2], f32, tag="mv")
                        nc.vector.bn_aggr(out=mv, in_=stats)
                        # rhs2 = [mean_c, E[x^2]_c]
                        rhs2 = gn.tile([P, 2], f32, tag="rhs2")
                        nc.vector.tensor_copy(out=rhs2[:, 0:1], in_=mv[:, 0:1])
                        nc.vector.scalar_tensor_tensor(
                            out=rhs2[:, 1:2], in0=mv[:, 0:1], scalar=mv[:, 0:1],
                            in1=mv[:, 1:2], op0=Alu.mult, op1=Alu.add,
                        )
                        # group sums over the 16 channels of each group
                        g_ps = gnps.tile([P, 2], f32, tag="g_ps")
                        nc.tensor.matmul(
                            out=g_ps[0:GPC, :], lhsT=selsum_sb, rhs=rhs2,
                            start=True, stop=True,
                        )
                        gs_t = gn.tile([P, 2], f32, tag="gs")
                        nc.vector.tensor_scalar(
                            out=gs_t[0:GPC, :], in0=g_ps[0:GPC, :],
                            scalar1=1.0 / GS, scalar2=None, op0=Alu.mult,
                        )
                        mean2 = gn.tile([P, 1], f32, tag="mean2")
                        nc.vector.tensor_mul(mean2[0:GPC], gs_t[0:GPC, 0:1],
                                             gs_t[0:GPC, 0:1])
                        var = gn.tile([P, 1], f32, tag="var")
                        nc.vector.tensor_sub(var[0:GPC], gs_t[0:GPC, 1:2],
                                             mean2[0:GPC])
                        sq = gn.tile([P, 1], f32, tag="sq")
                        nc.scalar.activation(
                            out=sq[0:GPC], in_=var[0:GPC], func=Act.Sqrt,
                            bias=eps_sb[0:GPC], scale=1.0,
                        )
                        gmr = gn.tile([P, 2], f32, tag="gmr")
                        nc.vector.tensor_copy(out=gmr[0:GPC, 0:1],
                                              in_=gs_t[0:GPC, 0:1])
                        nc.vector.reciprocal(out=gmr[0:GPC, 1:2], in_=sq[0:GPC])
                        # broadcast (mean_g, rstd_g) back to channels
                        bc_ps = gnps.tile([P, 2], f32, tag="bc_ps")
                        nc.tensor.matmul(
                            out=bc_ps, lhsT=selbc_sb[0:GPC, :], rhs=gmr[0:GPC, :],
                            start=True, stop=True,
                        )
                        a_t = gn.tile([P, 1], f32, tag="a")
                        nc.vector.tensor_mul(a_t, bc_ps[:, 1:2], gamma_sb[cc])
                        na_t = gn.tile([P, 1], f32, tag="na")
                        nc.vector.tensor_scalar_mul(na_t, a_t, -1.0)
                        b_t = gn.tile([P, 1], f32, tag="b")
                        nc.vector.scalar_tensor_tensor(
                            out=b_t, in0=bc_ps[:, 0:1], scalar=na_t,
                            in1=beta_sb[cc], op0=Alu.mult, op1=Alu.add,
                        )
                        # h = x*a + b -> fp8 pair slice; last chunk on DVE
                        # (shorter critical path into the QKV matmuls)
                        if cc < NCH - 1:
                            nc.scalar.activation(
                                out=h_sb[cc // 2][:, cc % 2, :], in_=xt,
                                func=Act.Identity, scale=a_t, bias=b_t,
                            )
                        else:
                            nc.vector.tensor_scalar(
                                out=h_sb[cc // 2][:, cc % 2, :], in0=xt,
                                scalar1=a_t, scalar2=b_t,
                                op0=Alu.mult, op1=Alu.add,
                            )

                # ---- QKV projections (all fp8 DoubleRow); PSUM->fp8
                # epilogues alternate ACT/DVE so neither engine gates PE ----
                with tc.tile_pool(name="qkvps", bufs=4, space="PSUM") as qkvps:
                    for w_sb, b_sb, dst in ((wq_sb, qb_sb, q_sb),
                                            (wk_sb, kb_sb, k_sb)):
                        for oc in range(NCH):
                            osl = slice(oc * P, (oc + 1) * P)
                            for nt in range(NB):
                                nsl = slice(nt * BW, (nt + 1) * BW)
                                pt = qkvps.tile([P, BW], f32, tag="qkv")
                                for p in range(NOP):
                                    nc.tensor.matmul(
                                        out=pt,
                                        lhsT=w_sb[p][:, 0:2, osl],
                                        rhs=h_sb[p][:, 0:2, nsl],
                                        start=(p == 0), stop=(p == NOP - 1),
                                        perf_mode=DR,
                                    )
                                if nt % 2 == 0:
                                    nc.scalar.activation(
                                        out=dst[oc // 2][:, oc % 2, nsl],
                                        in_=pt, func=Act.Identity,
                                        scale=QK_DSCALE, bias=b_sb[oc],
                                    )
                                else:
                                    nc.vector.tensor_scalar(
                                        out=dst[oc // 2][:, oc % 2, nsl],
                                        in0=pt, scalar1=QK_DSCALE,
                                        scalar2=b_sb[oc],
                                        op0=Alu.mult, op1=Alu.add,
                                    )
                    # vT[m, o] = sum_c h[c, m] wv[c, o]; v_b is folded into
                    # pb on the host (softmax weights sum to 1)
                    for mt in range(MT):
                        msl = slice(mt * P, (mt + 1) * P)
                        pt = qkvps.tile([P, BW], f32, tag="qkv")
                        for p in range(NOP):
                            nc.tensor.matmul(
                                out=pt, lhsT=h_sb[p][:, 0:2, msl],
                                rhs=wv_sb[p], start=(p == 0),
                                stop=(p == NOP - 1), perf_mode=DR,
                            )
                        if mt % 2 == 0:
                            nc.scalar.activation(
                                out=vt_sb[mt // 2][:, mt % 2, :], in_=pt,
                                func=Act.Identity, scale=V_DSCALE,
                            )
                        else:
                            nc.vector.tensor_scalar(
                                out=vt_sb[mt // 2][:, mt % 2, :], in0=pt,
                                scalar1=V_DSCALE, scalar2=None, op0=Alu.mult,
                            )

            # ---- attention (fp8 DR) + interleaved delayed proj + residual --
            with (
                tc.tile_pool(name="eps_ps", bufs=2, space="PSUM") as e_ps,
                tc.tile_pool(name="s_ps", bufs=1, space="PSUM") as s_ps,
                tc.tile_pool(name="h_ps", bufs=1, space="PSUM") as h_ps,
                tc.tile_pool(name="p_ps", bufs=1, space="PSUM") as p_ps,
                tc.tile_pool(name="expt", bufs=8) as expt,
                tc.tile_pool(name="sumt", bufs=3) as sumt,
                tc.tile_pool(name="hup", bufs=2) as hup,
                tc.tile_pool(name="epil", bufs=2) as epil,
            ):
                def emit_proj_group(nbp, hu, oc2, last=False):
                    nsl = slice(nbp * BW, (nbp + 1) * BW)
                    osl = slice(oc2 * P, (oc2 + 1) * P)
                    pp = p_ps.tile([P, BW], f32, tag="p", name="pp")
                    for p in range(NOP):
                        nc.tensor.matmul(
                            out=pp, lhsT=wp_sb[p][:, 0:2, osl], rhs=hu[p],
                            start=(p == 0), stop=(p == NOP - 1), perf_mode=DR,
                        )
                    xt = epil.tile([P, BW], f32, tag="xt", name="xt")
                    nc.sync.dma_start(out=xt, in_=x_d[osl, nsl])
                    # out = pp/32 + pb + x; the last block's scale+bias goes
                    # on ACT (idle after the final EXP) to shorten the tail
                    t1 = epil.tile([P, BW], f32, tag="t1", name="t1")
                    if last:
                        nc.scalar.activation(
                            out=t1, in_=pp, func=Act.Identity,
                            scale=P_DSCALE, bias=pb_sb[oc2],
                        )
                    else:
                        nc.vector.tensor_scalar(
                            out=t1, in0=pp, scalar1=P_DSCALE,
                            scalar2=pb_sb[oc2], op0=Alu.mult, op1=Alu.add,
                        )
                    ot = epil.tile([P, BW], f32, tag="ot", name="ot")
                    nc.vector.tensor_tensor(out=ot, in0=t1, in1=xt, op=Alu.add)
                    nc.sync.dma_start(out=out_d[osl, nsl], in_=ot)

                def emit_av(pr, et, ph):
                    """attn.v matmuls for one m-pair (consume et)."""
                    for oc in range(NCH):
                        nc.tensor.matmul(
                            out=ph[oc],
                            lhsT=vt_sb[pr][:, 0:2, oc * P : (oc + 1) * P],
                            rhs=et,
                            start=(pr == 0), stop=(pr == MPAIR - 1),
                            perf_mode=DR,
                        )

                def emit_ones(g, sg, ps_s, ng):
                    nc.tensor.matmul(
                        out=ps_s, lhsT=ones8_sb, rhs=sg,
                        start=(g == 0), stop=(g == ng - 1),
                        perf_mode=DR,
                    )

                pending = None
                for nb in range(NB):
                    nsl = slice(nb * BW, (nb + 1) * BW)
                    ps_s = s_ps.tile([P, BW], f32, tag="s", name="ps_s")
                    ph = [h_ps.tile([P, BW], f32, tag=f"h{oc}", name=f"hps{oc}")
                          for oc in range(NCH)]
                    # software pipeline: AV runs one m-pair behind the E
                    # matmuls so PE never waits on the EXP latency; the
                    # softmax denominator sums groups of 4 exp tiles on the
                    # (otherwise idle) GpSimd engine so only one ones-matmul
                    # per group hits the PE
                    # last block keeps the shallow 2-way sum (shorter
                    # dependency tail into the final reciprocal/proj)
                    four_way = nb < NB - 1
                    ets, sgs, roots = [], [], []
                    for pr in range(MPAIR):
                        et = expt.tile([P, 2, BW], fp8, tag="et", name="et")
                        for j in range(2):
                            mt = 2 * pr + j
                            msl = slice(mt * P, (mt + 1) * P)
                            pe = e_ps.tile([P, BW], f32, tag="e", name="pe")
                            for op in range(NOP):
                                nc.tensor.matmul(
                                    out=pe, lhsT=k_sb[op][:, 0:2, msl],
                                    rhs=q_sb[op][:, 0:2, nsl],
                                    start=(op == 0), stop=(op == NOP - 1),
                                    perf_mode=DR,
                                )
                            nc.scalar.activation(
                                out=et[:, j, :], in_=pe, func=Act.Exp,
                                bias=negc_sb, scale=EXP_SCALE,
                            )
                        ets.append(et)
                        if pr % 2 == 1:
                            # elementwise pair-sum (values <~50 << fp8 max)
                            sg = sumt.tile([P, 2, BW], fp8, tag="sg",
                                           name="sg")
                            nc.gpsimd.tensor_tensor(
                                out=sg, in0=ets[pr - 1], in1=ets[pr],
                                op=Alu.add,
                            )
                            sgs.append(sg)
                            if four_way and pr % 4 == 3:
                                rt = sumt.tile([P, 2, BW], fp8, tag="rt",
                                               name="rt")
                                nc.vector.tensor_tensor(
                                    out=rt, in0=sgs[-2], in1=sgs[-1],
                                    op=Alu.add,
                                )
                                roots.append(rt)
                        if pr > 0:
                            emit_av(pr - 1, ets[pr - 1], ph)
                            # spread prev block's proj through the m-loop
                            if pending is not None and pr % 4 == 0:
                                emit_proj_group(pending[0], pending[1],
                                                pr // 4 - 1)
                            # denominator matmuls trail their sums by 2-4
                            # m-pairs (hides the gpsimd/DVE add latency)
                            if four_way:
                                if pr % 4 == 3 and pr >= 7:
                                    emit_ones(pr // 4 - 1, roots[pr // 4 - 1],
                                              ps_s, MPAIR // 4)
                            elif pr % 2 == 1 and pr >= 3:
                                emit_ones(pr // 2 - 1, sgs[pr // 2 - 1],
                                          ps_s, MPAIR // 2)
                    emit_av(MPAIR - 1, ets[MPAIR - 1], ph)
                    if pending is not None:
                        emit_proj_group(pending[0], pending[1], NCH - 1)
                    if four_way:
                        emit_ones(MPAIR // 4 - 1, roots[-1], ps_s, MPAIR // 4)
                    else:
                        emit_ones(MPAIR // 2 - 1, sgs[-1], ps_s, MPAIR // 2)
                    # r = 4/S (ones are 0.25s); normalize into fp8 pairs on
                    # DVE (keeps ACT pure-EXP)
                    r_t = epil.tile([P, BW], f32, tag="r", name="r_t")
                    nc.vector.reciprocal_approx_fast(out=r_t, in_=ps_s)
                    hu = [hup.tile([P, 2, BW], fp8, tag=f"hu{p}",
                                   name=f"hu{p}") for p in range(NOP)]
                    for oc in range(NCH):
                        nc.vector.tensor_tensor(
                            out=hu[oc // 2][:, oc % 2, :], in0=ph[oc],
                            in1=r_t, op=Alu.mult,
                        )
                    pending = (nb, hu)
                for oc2 in range(NCH):
                    emit_proj_group(pending[0], pending[1], oc2, last=True)

    nc.compile()
    return nc


def _build_exec():
    import jax
    from jax.experimental.shard_map import shard_map
    from jax.sharding import Mesh, PartitionSpec

    from concourse import bass2jax, mybir

    nc = _build_nc()
    bass2jax.install_neuronx_cc_hook()

    partition_name = (
        nc.partition_id_tensor.name if nc.partition_id_tensor else None
    )
    in_names, out_names, out_avals = [], [], []
    for alloc in nc.m.functions[0].allocations:
        if not isinstance(alloc, mybir.MemoryLocationSet):
            continue
        name = alloc.memorylocations[0].name
        if alloc.kind == "ExternalInput":
            if name != partition_name:
                in_names.append(name)
        elif alloc.kind == "ExternalOutput":
            out_names.append(name)
            out_avals.append(
                jax.core.ShapedArray(
                    tuple(alloc.tensor_shape), mybir.dt.np(alloc.dtype)
                )
            )
    n_params = len(in_names)
    all_in = tuple(in_names + out_names)
    if partition_name is not None:
        all_in = all_in + (partition_name,)
    donate = tuple(range(n_params, n_params + len(out_names)))

    def _body(*args):
        operands = list(args)
        if partition_name is not None:
            operands.append(bass2jax.partition_id_tensor())
        outs = bass2jax._bass_exec_p.bind(
            *operands,
            out_avals=tuple(out_avals),
            in_names=all_in,
            out_names=tuple(out_names),
            lowering_input_output_aliases=(),
            sim_require_finite=True,
            sim_require_nnan=True,
            nc=nc,
        )
        return tuple(outs)

    devices = jax.devices()[:NCORES]
    mesh = Mesh(np.asarray(devices), ("core",))
    in_specs = (PartitionSpec("core"),) * (n_params + len(out_names))
    out_specs = (PartitionSpec("core"),) * len(out_names)
    sharded = jax.jit(
        shard_map(
            _body, mesh=mesh, in_specs=in_specs, out_specs=out_specs,
            check_rep=False,
        ),
        donate_argnums=donate,
        keep_unused=True,
    )
    return sharded, in_names, out_names, out_avals, nc


def _get_exec():
    global _EXEC
    if _EXEC is None:
        _EXEC = _build_exec()
    return _EXEC


def _selsum():
    s = np.zeros((P, GPC), np.float32)
    s[np.arange(P), np.arange(P) // GS] = 1.0
    return s


def _pair_fp8(w, boost):
    """[C_out, C_in] torch-style weight -> fp8 DR pair layout [NOP,P,2,C]."""
    wt = np.ascontiguousarray(np.asarray(w, np.float32).T) * boost
    return np.ascontiguousarray(
        wt.reshape(NOP, 2, P, C).transpose(0, 2, 1, 3)
    ).astype(FP8)


def make_concat_inputs(inputs):
    """Host-side prep: per-core shards concatenated on axis 0 (shard_map)."""
    x = np.asarray(inputs["x"], np.float32).reshape(B, C, N)
    sqs = np.float32(C ** -0.25)

    shared = {
        "wq": _pair_fp8(inputs["q_w"], sqs * QK_WBOOST),
        "wk": _pair_fp8(inputs["k_w"], sqs * QK_WBOOST),
        "wv": _pair_fp8(inputs["v_w"], V_WBOOST),
        "wp": _pair_fp8(inputs["proj_w"], P_WBOOST),
        # [gamma, beta, qb', kb', pb'] columns; pb' folds in proj_w @ v_b
        # (softmax weights sum to 1, so v_b is a constant proj-side shift)
        "vecs": np.ascontiguousarray(np.stack([
            np.asarray(inputs["gamma"], np.float32),
            np.asarray(inputs["beta"], np.float32),
            np.asarray(inputs["q_b"], np.float32)
            * (sqs * QK_WBOOST * QK_DSCALE),
            np.asarray(inputs["k_b"], np.float32)
            * (sqs * QK_WBOOST * QK_DSCALE),
            np.asarray(inputs["proj_b"], np.float32)
            + np.asarray(inputs["proj_w"], np.float32)
            @ np.asarray(inputs["v_b"], np.float32),
        ], axis=1)),
        "selsum": _selsum(),
        "selbc": np.ascontiguousarray(_selsum().T),
        "ones8": np.full((P, 2 * P), ONES_VAL, FP8),
    }
    per_core = [dict(shared, x=np.ascontiguousarray(x[c]))
                for c in range(NCORES)]

    sharded, in_names, out_names, out_avals, _ = _get_exec()
    concat_in = [
        np.concatenate([per_core[c][nm] for c in range(NCORES)], axis=0)
        for nm in in_names
    ]
    return concat_in, out_avals


def run_concat(concat_in, out_avals):
    sharded = _get_exec()[0]
    concat_zeros = [
        np.zeros((NCORES * av.shape[0], *av.shape[1:]), av.dtype)
        for av in out_avals
    ]
    outs = sharded(*concat_in, *concat_zeros)
    return outs


def kernel(**inputs):
    concat_in, out_avals = make_concat_inputs(inputs)
    outs = run_concat(concat_in, out_avals)
    o = np.asarray(outs[0]).reshape(NCORES, C, N)
    return np.ascontiguousarray(o.reshape(B, C, H, W), dtype=np.float32)


# revision 23
# speedup vs baseline: 1.2246x; 1.2085x over previous
"""Trainium2 Bass kernel for nn_AttentionBlock (GroupNorm -> QKV 1x1 -> softmax
attention over 4096 tokens -> proj + residual).

Sharding: pure data-parallel over batch B=8 across the 8 NeuronCores (one
batch element per core); attention is per-batch-element so no collectives.

Per-core layout (C=512 channels, N=4096 tokens), all matmuls fp8e4 DoubleRow:
  - x loaded [channel-part, token] as 4 chunks of [128, 4096], DMA'd per
    512-col slice FIRST (before weights) so GroupNorm stats pipeline behind
    the DMA instead of serializing after it
  - GroupNorm fp32 (bn_stats per slice; group reduce/broadcast via tiny
    matmuls); h stored fp8 in DoubleRow pair layout [128, 2, 4096]
  - weights pre-scaled by powers of 2 so every fp8 tensor sits in e4m3's
    sweet spot; compensation folded into activation scales:
      wq,wk *= C^-0.25 * 32 (q,k stored as q*C^-0.25*4 -> logits' = 16*logit,
      EXP uses scale=1/16); wv *= 8 (v epilogue scale 1/8); wp *= 8
  - logits computed transposed: E^T[m, n] = sum_o k[o,m] q[o,n]; softmax via
    exp(logit - 2.5) in fp8 (shift cancels in normalization); denominator
    via a 0.25-valued ones-matmul so reciprocal gives 4/S directly
  - h_attn_unnorm[o, n] = sum_m vT[m, o] expT[m, n]; normalized by 4/S on
    the Vector engine into fp8 pairs (keeps Scalar engine pure-EXP during
    attention -> no ACT table thrash), then proj as fp8 DR one n-block
    behind, its 4 output groups interleaved into the next block's m-loop
  - final: out = pp/32 + pb + x (residual re-DMA'd, overlapped)

Self-contained: hardcodes shapes; builds + compiles the Bass graph once and
caches a persistent jitted shard_map executor over the 8 axon NeuronCores.
"""

import os
import sys

sys.path.insert(0, "/opt/trn_rl_repo")
os.environ.setdefault("MYCRO_LOCAL_CACHE", "1")

import numpy as np
import ml_dtypes

BF16 = ml_dtypes.bfloat16
FP8 = ml_dtypes.float8_e4m3

# Problem constants (hardcoded; kernel.py must not read spec/reference files)
B, C, H, W = 8, 512, 64, 64
N = H * W            # 4096 tokens
P = 128              # partitions
NCH = C // P         # 4 channel chunks
NOP = NCH // 2       # 2 channel-chunk pairs (DoubleRow)
BW = 512             # n-block width (= PSUM bank in fp32)
NB = N // BW         # 8 n-blocks
MT = N // P          # 32 m-tiles
MPAIR = MT // 2      # 16 m-tile pairs (DoubleRow)
G = 32               # groups
GS = C // G          # 16 channels per group
GPC = P // GS        # 8 groups per 128-channel chunk
NSG = 8              # GN stat slices per chunk (512 cols each)
EPS = 1e-6
EXP_SHIFT = 2.5      # exp(logit - shift); cancels in softmax normalization
NCORES = 8

# fp8 scale management (powers of 2; see module docstring)
QK_WBOOST = 32.0     # wq,wk stored * C^-0.25 * 32
QK_DSCALE = 0.125    # q epilogue: q' = psum/8 + qb'  (q' = q * C^-0.25 * 4)
EXP_SCALE = 0.0625   # logits' = 16 * logits
V_WBOOST = 8.0       # wv stored * 8
V_DSCALE = 0.125     # v epilogue: v' = psum/8 + vb  (v' = v)
P_WBOOST = 8.0       # wp stored * 8
ONES_VAL = 0.25      # ps_s = S/4 -> r = 4/S; hu = h_unnorm * 4/S
P_DSCALE = 1.0 / 32  # out = psum/32 + pb + x  (psum = 8*wp . 4*h_norm)

_EXEC = None


def _build_nc():
    import concourse.bacc as bacc
    import concourse.tile as tile
    from concourse import mybir

    f32 = mybir.dt.float32
    fp8 = mybir.dt.float8e4
    Alu = mybir.AluOpType
    Act = mybir.ActivationFunctionType
    DR = mybir.MatmulPerfMode.DoubleRow

    nc = bacc.Bacc(
        "TRN2", target_bir_lowering=False, debug=False, num_devices=NCORES
    )

    def din(name, shape, dt=f32):
        return nc.declare_dram_parameter(name, list(shape), dt, isOutput=False)

    x_d = din("x", [C, N])
    wq_d = din("wq", [NOP, P, 2, C], fp8)   # pair layout, pre-scaled
    wk_d = din("wk", [NOP, P, 2, C], fp8)
    wv_d = din("wv", [NOP, P, 2, C], fp8)
    wp_d = din("wp", [NOP, P, 2, C], fp8)
    # [gamma, beta, qb, kb, pb] columns; pb has proj_w @ v_b folded in
    # (softmax sums to 1) -> one DMA for all per-channel vectors
    vecs_d = din("vecs", [C, 5])
    selsum_d = din("selsum", [P, GPC])
    selbc_d = din("selbc", [GPC, P])
    ones8_d = din("ones8", [P, 2 * P], fp8)   # DoubleRow 0.25s [P, 2, P]
    out_d = nc.declare_dram_parameter("out", [C, N], f32, isOutput=True)

    with tile.TileContext(nc) as tc:
        with (
            tc.tile_pool(name="consts", bufs=1) as consts,
            tc.tile_pool(name="qksb", bufs=1) as qkp,
            tc.tile_pool(name="vtsb", bufs=1) as vtp,
            tc.tile_pool(name="xsb", bufs=1) as xp,
        ):
            # ---- small consts first (GroupNorm needs them immediately);
            # batched into 4 DMAs (each dma_start costs ~650ns of sequencer
            # descriptor-gen, so fewer+bigger wins) ----
            vecs_sb = consts.tile([P, NCH, 5], f32, tag="vecs")
            nc.sync.dma_start(
                out=vecs_sb,
                in_=vecs_d[:, :].rearrange("(c p) v -> p c v", p=P),
            )
            gamma_sb = [vecs_sb[:, cc, 0:1] for cc in range(NCH)]
            beta_sb = [vecs_sb[:, cc, 1:2] for cc in range(NCH)]
            qb_sb = [vecs_sb[:, cc, 2:3] for cc in range(NCH)]
            kb_sb = [vecs_sb[:, cc, 3:4] for cc in range(NCH)]
            pb_sb = [vecs_sb[:, cc, 4:5] for cc in range(NCH)]
            selsum_sb = consts.tile([P, GPC], f32, tag="selsum")
            nc.sync.dma_start(out=selsum_sb, in_=selsum_d[:, :])
            selbc_sb = consts.tile([P, P], f32, tag="selbc")
            nc.sync.dma_start(out=selbc_sb[0:GPC, :], in_=selbc_d[:, :])
            ones8_sb = consts.tile([P, 2, P], fp8, tag="ones8")
            nc.sync.dma_start(
                out=ones8_sb,
                in_=ones8_d[:, :].rearrange("p (j q) -> p j q", j=2),
            )
            eps_sb = consts.tile([P, 1], f32, tag="eps")
            nc.vector.memset(eps_sb, EPS)
            negc_sb = consts.tile([P, 1], f32, tag="negc")
            nc.vector.memset(negc_sb, -EXP_SHIFT)

            # ---- x DMAs next (GN critical path); half-chunk transfers
            # (finer arrival granularity for the bn_stats pipeline) ----
            x_sb = []
            for cc in range(NCH):
                xt = xp.tile([P, N], f32, tag=f"x{cc}", name=f"x{cc}")
                for hh in range(2):
                    nsl = slice(hh * (N // 2), (hh + 1) * (N // 2))
                    nc.sync.dma_start(
                        out=xt[:, nsl], in_=x_d[cc * P : (cc + 1) * P, nsl]
                    )
                x_sb.append(xt)

            # ---- weights on the gpsimd queue (concurrent with x; only
            # 1 MiB so they land early without delaying the x stream) ----
            def wpairs(d, tagp):
                ts = []
                for p in range(NOP):
                    t = consts.tile([P, 2, C], fp8, tag=f"{tagp}{p}",
                                    name=f"{tagp}{p}")
                    nc.gpsimd.dma_start(out=t, in_=d[p, :, :, :])
                    ts.append(t)
                return ts

            wq_sb = wpairs(wq_d, "wq")
            wk_sb = wpairs(wk_d, "wk")
            wv_sb = wpairs(wv_d, "wv")
            wp_sb = wpairs(wp_d, "wp")

            # q/k in DoubleRow pair layout: [P, 2, N], dim1 = pair member j,
            # channel chunk oc = 2*op + j; vT pairs [token-part, 2, C]
            q_sb = [qkp.tile([P, 2, N], fp8, tag=f"q{op}", name=f"q{op}")
                    for op in range(NOP)]
            k_sb = [qkp.tile([P, 2, N], fp8, tag=f"k{op}", name=f"k{op}")
                    for op in range(NOP)]
            vt_sb = [vtp.tile([P, 2, C], fp8, tag=f"vt{t}", name=f"vt{t}")
                     for t in range(MPAIR)]

            with tc.tile_pool(name="hsb", bufs=1) as hp:
                # h in fp8 pair layout for DR QKV matmuls
                h_sb = [hp.tile([P, 2, N], fp8, tag=f"h{p}", name=f"h{p}")
                        for p in range(NOP)]
                # ---- GroupNorm (per 128-channel chunk; groups don't cross) --
                with (
                    tc.tile_pool(name="gn", bufs=2) as gn,
                    tc.tile_pool(name="gnps", bufs=2, space="PSUM") as gnps,
                ):
                    for cc in range(NCH):
                        xt = x_sb[cc]
                        stats = gn.tile([P, NSG, 6], f32, tag="stats")
                        sw = N // NSG
                        for sg in range(NSG):
                            nc.vector.bn_stats(
                                out=stats[:, sg, :],
                                in_=xt[:, sg * sw : (sg + 1) * sw],
                            )
                        mv = gn.tile([P, 2], f32, tag="mv")
                        nc.vector.bn_aggr(out=mv, in_=stats)
                        # rhs2 = [mean_c, E[x^2]_c]
                        rhs2 = gn.tile([P, 2], f32, tag="rhs2")
                        nc.vector.tensor_copy(out=rhs2[:, 0:1], in_=mv[:, 0:1])
                        nc.vector.scalar_tensor_tensor(
                            out=rhs2[:, 1:2], in0=mv[:, 0:1], scalar=mv[:, 0:1],
                            in1=mv[:, 1:2], op0=Alu.mult, op1=Alu.add,
                        )
                        # group sums over the 16 channels of each group
                        g_ps = gnps.tile([P, 2], f32, tag="g_ps")
                        nc.tensor.matmul(
                            out=g_ps[0:GPC, :], lhsT=selsum_sb, rhs=rhs2,
                            start=True, stop=True,
                        )
                        gs_t = gn.tile([P, 2], f32, tag="gs")
                        nc.vector.tensor_scalar(
                            out=gs_t[0:GPC, :], in0=g_ps[0:GPC, :],
                            scalar1=1.0 / GS, scalar2=None, op0=Alu.mult,
                        )
                        mean2 = gn.tile([P, 1], f32, tag="mean2")
                        nc.vector.tensor_mul(mean2[0:GPC], gs_t[0:GPC, 0:1],
                                             gs_t[0:GPC, 0:1])
                        var = gn.tile([P, 1], f32, tag="var")
                        nc.vector.tensor_sub(var[0:GPC], gs_t[0:GPC, 1:2],
                                             mean2[0:GPC])
                        sq = gn.tile([P, 1], f32, tag="sq")
                        nc.scalar.activation(
                            out=sq[0:GPC], in_=var[0:GPC], func=Act.Sqrt,
                            bias=eps_sb[0:GPC], scale=1.0,
                        )
                        gmr = gn.tile([P, 2], f32, tag="gmr")
                        nc.vector.tensor_copy(out=gmr[0:GPC, 0:1],
                                              in_=gs_t[0:GPC, 0:1])
                        nc.vector.reciprocal(out=gmr[0:GPC, 1:2], in_=sq[0:GPC])
                        # broadcast (mean_g, rstd_g) back to channels
                        bc_ps = gnps.tile([P, 2], f32, tag="bc_ps")
                        nc.tensor.matmul(
                            out=bc_ps, lhsT=selbc_sb[0:GPC, :], rhs=gmr[0:GPC, :],
                            start=True, stop=True,
                        )
                        a_t = gn.tile([P, 1], f32, tag="a")
                        nc.vector.tensor_mul(a_t, bc_ps[:, 1:2], gamma_sb[cc])
                        na_t = gn.tile([P, 1], f32, tag="na")
                        nc.vector.tensor_scalar_mul(na_t, a_t, -1.0)
                        b_t = gn.tile([P, 1], f32, tag="b")
                        nc.vector.scalar_tensor_tensor(
                            out=b_t, in0=bc_ps[:, 0:1], scalar=na_t,
                            in1=beta_sb[cc], op0=Alu.mult, op1=Alu.add,
                        )
                        # h = x*a + b -> fp8 pair slice; last chunk on DVE
                        # (shorter critical path into the QKV matmuls)
                        if cc < NCH - 1:
                            nc.scalar.activation(
                                out=h_sb[cc // 2][:, cc % 2, :], in_=xt,
                                func=Act.Identity, scale=a_t, bias=b_t,
                            )
                        else:
                            nc.vector.tensor_scalar(
                                out=h_sb[cc // 2][:, cc % 2, :], in0=xt,
                                scalar1=a_t, scalar2=b_t,
                                op0=Alu.mult, op1=Alu.add,
                            )

                # ---- QKV projections (all fp8 DoubleRow); PSUM->fp8
                # epilogues alternate ACT/DVE so neither engine gates PE ----
                with tc.tile_pool(name="qkvps", bufs=4, space="PSUM") as qkvps:
                    for w_sb, b_sb, dst in ((wq_sb, qb_sb, q_sb),
                                            (wk_sb, kb_sb, k_sb)):
                        for oc in range(NCH):
                            osl = slice(oc * P, (oc + 1) * P)
                            for nt in range(NB):
                                nsl = slice(nt * BW, (nt + 1) * BW)
                                pt = qkvps.tile([P, BW], f32, tag="qkv")
                                for p in range(NOP):
                                    nc.tensor.matmul(
                                        out=pt,
                                        lhsT=w_sb[p][:, 0:2, osl],
                                        rhs=h_sb[p][:, 0:2, nsl],
                                        start=(p == 0), stop=(p == NOP - 1),
                                        perf_mode=DR,
                                    )
                                if nt % 2 == 0:
                                    nc.scalar.activation(
                                        out=dst[oc // 2][:, oc % 2, nsl],
                                        in_=pt, func=Act.Identity,
                                        scale=QK_DSCALE, bias=b_sb[oc],
                                    )
                                else:
                                    nc.vector.tensor_scalar(
                                        out=dst[oc // 2][:, oc % 2, nsl],
                                        in0=pt, scalar1=QK_DSCALE,
                                        scalar2=b_sb[oc],
                                        op0=Alu.mult, op1=Alu.add,
                                    )
                    # vT[m, o] = sum_c h[c, m] wv[c, o]; v_b is folded into
                    # pb on the host (softmax weights sum to 1)
                    for mt in range(MT):
                        msl = slice(mt * P, (mt + 1) * P)
                        pt = qkvps.tile([P, BW], f32, tag="qkv")
                        for p in range(NOP):
                            nc.tensor.matmul(
                                out=pt, lhsT=h_sb[p][:, 0:2, msl],
                                rhs=wv_sb[p], start=(p == 0),
                                stop=(p == NOP - 1), perf_mode=DR,
                            )
                        if mt % 2 == 0:
                            nc.scalar.activation(
                                out=vt_sb[mt // 2][:, mt % 2, :], in_=pt,
                                func=Act.Identity, scale=V_DSCALE,
                            )
                        else:
                            nc.vector.tensor_scalar(
                                out=vt_sb[mt // 2][:, mt % 2, :], in0=pt,
                                scalar1=V_DSCALE, scalar2=None, op0=Alu.mult,
                            )

            # ---- attention (fp8 DR) + interleaved delayed proj + residual --
            with (
                tc.tile_pool(name="eps_ps", bufs=2, space="PSUM") as e_ps,
                tc.tile_pool(name="s_ps", bufs=1, space="PSUM") as s_ps,
                tc.tile_pool(name="h_ps", bufs=1, space="PSUM") as h_ps,
                tc.tile_pool(name="p_ps", bufs=1, space="PSUM") as p_ps,
                tc.tile_pool(name="expt", bufs=8) as expt,
                tc.tile_pool(name="sumt", bufs=3) as sumt,
                tc.tile_pool(name="hup", bufs=2) as hup,
                tc.tile_pool(name="epil", bufs=2) as epil,
            ):
                def emit_proj_group(nbp, hu, oc2, last=False):
                    nsl = slice(nbp * BW, (nbp + 1) * BW)
                    osl = slice(oc2 * P, (oc2 + 1) * P)
                    pp = p_ps.tile([P, BW], f32, tag="p", name="pp")
                    for p in range(NOP):
                        nc.tensor.matmul(
                            out=pp, lhsT=wp_sb[p][:, 0:2, osl], rhs=hu[p],
                            start=(p == 0), stop=(p == NOP - 1), perf_mode=DR,
                        )
                    xt = epil.tile([P, BW], f32, tag="xt", name="xt")
                    nc.sync.dma_start(out=xt, in_=x_d[osl, nsl])
                    # out = pp/32 + pb + x; the last block's scale+bias goes
                    # on ACT (idle after the final EXP) to shorten the tail
                    t1 = epil.tile([P, BW], f32, tag="t1", name="t1")
                    if last:
                        nc.scalar.activation(
                            out=t1, in_=pp, func=Act.Identity,
                            scale=P_DSCALE, bias=pb_sb[oc2],
                        )
                    else:
                        nc.vector.tensor_scalar(
                            out=t1, in0=pp, scalar1=P_DSCALE,
                            scalar2=pb_sb[oc2], op0=Alu.mult, op1=Alu.add,
                        )
                    ot = epil.tile([P, BW], f32, tag="ot", name="ot")
                    nc.vector.tensor_tensor(out=ot, in0=t1, in1=xt, op=Alu.add)
                    nc.sync.dma_start(out=out_d[osl, nsl], in_=ot)

                def emit_av(pr, et, ph):
                    """attn.v matmuls for one m-pair (consume et)."""
                    for oc in range(NCH):
                        nc.tensor.matmul(
                            out=ph[oc],
                            lhsT=vt_sb[pr][:, 0:2, oc * P : (oc + 1) * P],
                            rhs=et,
                            start=(pr == 0), stop=(pr == MPAIR - 1),
                            perf_mode=DR,
                        )

                def emit_ones(g, sg, ps_s, ng):
                    nc.tensor.matmul(
                        out=ps_s, lhsT=ones8_sb, rhs=sg,
                        start=(g == 0), stop=(g == ng - 1),
                        perf_mode=DR,
                    )

                pending = None
                for nb in range(NB):
                    nsl = slice(nb * BW, (nb + 1) * BW)
                    ps_s = s_ps.tile([P, BW], f32, tag="s", name="ps_s")
                    ph = [h_ps.tile([P, BW], f32, tag=f"h{oc}", name=f"hps{oc}")
                          for oc in range(NCH)]
                    # software pipeline: AV runs one m-pair behind the E
                    # matmuls so PE never waits on the EXP latency; the
                    # softmax denominator sums groups of 4 exp tiles on the
                    # (otherwise idle) GpSimd engine so only one ones-matmul
                    # per group hits the PE
                    ets, sgs, roots = [], [], []
                    for pr in range(MPAIR):
                        et = expt.tile([P, 2, BW], fp8, tag="et", name="et")
                        for j in range(2):
                            mt = 2 * pr + j
                            msl = slice(mt * P, (mt + 1) * P)
                            pe = e_ps.tile([P, BW], f32, tag="e", name="pe")
                            for op in range(NOP):
                                nc.tensor.matmul(
                                    out=pe, lhsT=k_sb[op][:, 0:2, msl],
                                    rhs=q_sb[op][:, 0:2, nsl],
                                    start=(op == 0), stop=(op == NOP - 1),
                                    perf_mode=DR,
                                )
                            nc.scalar.activation(
                                out=et[:, j, :], in_=pe, func=Act.Exp,
                                bias=negc_sb, scale=EXP_SCALE,
                            )
                        ets.append(et)
                        # denominator pre-sums: prs 0-11 in three 4-way
                        # groups (gpsimd pair-sums + DVE root), prs 12-13 a
                        # single 2-way, prs 14-15 consumed directly by the
                        # ones-matmul (keeps the block-boundary chain free
                        # of add latency); values <~100 << fp8 max 240
                        if pr % 2 == 1 and pr <= 13:
                            sg = sumt.tile([P, 2, BW], fp8, tag="sg",
                                           name="sg")
                            nc.gpsimd.tensor_tensor(
                                out=sg, in0=ets[pr - 1], in1=ets[pr],
                                op=Alu.add,
                            )
                            sgs.append(sg)
                            if pr % 4 == 3:
                                rt = sumt.tile([P, 2, BW], fp8, tag="rt",
                                               name="rt")
                                nc.vector.tensor_tensor(
                                    out=rt, in0=sgs[-2], in1=sgs[-1],
                                    op=Alu.add,
                                )
                                roots.append(rt)
                        if pr > 0:
                            emit_av(pr - 1, ets[pr - 1], ph)
                            # spread prev block's proj through the m-loop
                            if pending is not None and pr % 4 == 0:
                                emit_proj_group(pending[0], pending[1],
                                                pr // 4 - 1)
                            # root ones-matmuls trail their sums by 4 m-pairs
                            if pr % 4 == 3 and 7 <= pr <= 15:
                                emit_ones(pr // 4 - 1, roots[pr // 4 - 1],
                                          ps_s, 6)
                    emit_av(MPAIR - 1, ets[MPAIR - 1], ph)
                    if pending is not None:
                        emit_proj_group(pending[0], pending[1], NCH - 1)
                    emit_ones(3, sgs[6], ps_s, 6)
                    emit_ones(4, ets[14], ps_s, 6)
                    emit_ones(5, ets[15], ps_s, 6)
                    # r = 4/S (ones are 0.25s); normalize into fp8 pairs on
                    # DVE (keeps ACT pure-EXP)
                    r_t = epil.tile([P, BW], f32, tag="r", name="r_t")
                    nc.vector.reciprocal_approx_fast(out=r_t, in_=ps_s)
                    hu = [hup.tile([P, 2, BW], fp8, tag=f"hu{p}",
                                   name=f"hu{p}") for p in range(NOP)]
                    for oc in range(NCH):
                        nc.vector.tensor_tensor(
                            out=hu[oc // 2][:, oc % 2, :], in0=ph[oc],
                            in1=r_t, op=Alu.mult,
                        )
                    pending = (nb, hu)
                for oc2 in range(NCH):
                    emit_proj_group(pending[0], pending[1], oc2, last=True)

    nc.compile()
    return nc


def _build_exec():
    import jax
    from jax.experimental.shard_map import shard_map
    from jax.sharding import Mesh, PartitionSpec

    from concourse import bass2jax, mybir

    nc = _build_nc()
    bass2jax.install_neuronx_cc_hook()

    partition_name = (
        nc.partition_id_tensor.name if nc.partition_id_tensor else None
    )
    in_names, out_names, out_avals = [], [], []
    for alloc in nc.m.functions[0].allocations:
        if not isinstance(alloc, mybir.MemoryLocationSet):
            continue
        name = alloc.memorylocations[0].name
        if alloc.kind == "ExternalInput":
            if name != partition_name:
                in_names.append(name)
        elif alloc.kind == "ExternalOutput":
            out_names.append(name)
            out_avals.append(
                jax.core.ShapedArray(
                    tuple(alloc.tensor_shape), mybir.dt.np(alloc.dtype)
                )
            )
    n_params = len(in_names)
    all_in = tuple(in_names + out_names)
    if partition_name is not None:
        all_in = all_in + (partition_name,)
    donate = tuple(range(n_params, n_params + len(out_names)))

    def _body(*args):
        operands = list(args)
        if partition_name is not None:
            operands.append(bass2jax.partition_id_tensor())
        outs = bass2jax._bass_exec_p.bind(
            *operands,
            out_avals=tuple(out_avals),
            in_names=all_in,
            out_names=tuple(out_names),
            lowering_input_output_aliases=(),
            sim_require_finite=True,
            sim_require_nnan=True,
            nc=nc,
        )
        return tuple(outs)

    devices = jax.devices()[:NCORES]
    mesh = Mesh(np.asarray(devices), ("core",))
    in_specs = (PartitionSpec("core"),) * (n_params + len(out_names))
    out_specs = (PartitionSpec("core"),) * len(out_names)
    sharded = jax.jit(
        shard_map(
            _body, mesh=mesh, in_specs=in_specs, out_specs=out_specs,
            check_rep=False,
        ),
        donate_argnums=donate,
        keep_unused=True,
    )
    return sharded, in_names, out_names, out_avals, nc


def _get_exec():
    global _EXEC
    if _EXEC is None:
        _EXEC = _build_exec()
    return _EXEC


def _selsum():
    s = np.zeros((P, GPC), np.float32)
    s[np.arange(P), np.arange(P) // GS] = 1.0
    return s


def _pair_fp8(w, boost):
    """[C_out, C_in] torch-style weight -> fp8 DR pair layout [NOP,P,2,C]."""
    wt = np.ascontiguousarray(np.asarray(w, np.float32).T) * boost
    return np.ascontiguousarray(
        wt.reshape(NOP, 2, P, C).transpose(0, 2, 1, 3)
    ).astype(FP8)


def make_concat_inputs(inputs):
    """Host-side prep: per-core shards concatenated on axis 0 (shard_map)."""
    x = np.asarray(inputs["x"], np.float32).reshape(B, C, N)
    sqs = np.float32(C ** -0.25)

    shared = {
        "wq": _pair_fp8(inputs["q_w"], sqs * QK_WBOOST),
        "wk": _pair_fp8(inputs["k_w"], sqs * QK_WBOOST),
        "wv": _pair_fp8(inputs["v_w"], V_WBOOST),
        "wp": _pair_fp8(inputs["proj_w"], P_WBOOST),
        # [gamma, beta, qb', kb', pb'] columns; pb' folds in proj_w @ v_b
        # (softmax weights sum to 1, so v_b is a constant proj-side shift)
        "vecs": np.ascontiguousarray(np.stack([
            np.asarray(inputs["gamma"], np.float32),
            np.asarray(inputs["beta"], np.float32),
            np.asarray(inputs["q_b"], np.float32)
            * (sqs * QK_WBOOST * QK_DSCALE),
            np.asarray(inputs["k_b"], np.float32)
            * (sqs * QK_WBOOST * QK_DSCALE),
            np.asarray(inputs["proj_b"], np.float32)
            + np.asarray(inputs["proj_w"], np.float32)
            @ np.asarray(inputs["v_b"], np.float32),
        ], axis=1)),
        "selsum": _selsum(),
        "selbc": np.ascontiguousarray(_selsum().T),
        "ones8": np.full((P, 2 * P), ONES_VAL, FP8),
    }
    per_core = [dict(shared, x=np.ascontiguousarray(x[c]))
                for c in range(NCORES)]

    sharded, in_names, out_names, out_avals, _ = _get_exec()
    concat_in = [
        np.concatenate([per_core[c][nm] for c in range(NCORES)], axis=0)
        for nm in in_names
    ]
    return concat_in, out_avals


def run_concat(concat_in, out_avals):
    sharded = _get_exec()[0]
    concat_zeros = [
        np.zeros((NCORES * av.shape[0], *av.shape[1:]), av.dtype)
        for av in out_avals
    ]
    outs = sharded(*concat_in, *concat_zeros)
    return outs


def kernel(**inputs):
    concat_in, out_avals = make_concat_inputs(inputs)
    outs = run_concat(concat_in, out_avals)
    o = np.asarray(outs[0]).reshape(NCORES, C, N)
    return np.ascontiguousarray(o.reshape(B, C, H, W), dtype=np.float32)


# revision 26
# speedup vs baseline: 1.2449x; 1.0165x over previous
"""Trainium2 Bass kernel for nn_AttentionBlock (GroupNorm -> QKV 1x1 -> softmax
attention over 4096 tokens -> proj + residual).

Sharding: pure data-parallel over batch B=8 across the 8 NeuronCores (one
batch element per core); attention is per-batch-element so no collectives.

Per-core layout (C=512 channels, N=4096 tokens), all matmuls fp8e4 DoubleRow:
  - x loaded [channel-part, token] as 4 chunks of [128, 4096], DMA'd per
    512-col slice FIRST (before weights) so GroupNorm stats pipeline behind
    the DMA instead of serializing after it
  - GroupNorm fp32 (bn_stats per slice; group reduce/broadcast via tiny
    matmuls); h stored fp8 in DoubleRow pair layout [128, 2, 4096]
  - weights pre-scaled by powers of 2 so every fp8 tensor sits in e4m3's
    sweet spot; compensation folded into activation scales:
      wq,wk *= C^-0.25 * 32 (q,k stored as q*C^-0.25*4 -> logits' = 16*logit,
      EXP uses scale=1/16); wv *= 8 (v epilogue scale 1/8); wp *= 8
  - logits computed transposed: E^T[m, n] = sum_o k[o,m] q[o,n]; softmax via
    exp(logit - 2.5) in fp8 (shift cancels in normalization); denominator
    via a 0.25-valued ones-matmul so reciprocal gives 4/S directly
  - h_attn_unnorm[o, n] = sum_m vT[m, o] expT[m, n]; normalized by 4/S on
    the Vector engine into fp8 pairs (keeps Scalar engine pure-EXP during
    attention -> no ACT table thrash), then proj as fp8 DR one n-block
    behind, its 4 output groups interleaved into the next block's m-loop
  - final: out = pp/32 + pb + x (residual re-DMA'd, overlapped)

Self-contained: hardcodes shapes; builds + compiles the Bass graph once and
caches a persistent jitted shard_map executor over the 8 axon NeuronCores.
"""

import os
import sys

sys.path.insert(0, "/opt/trn_rl_repo")
os.environ.setdefault("MYCRO_LOCAL_CACHE", "1")

import numpy as np
import ml_dtypes

BF16 = ml_dtypes.bfloat16
FP8 = ml_dtypes.float8_e4m3

# Problem constants (hardcoded; kernel.py must not read spec/reference files)
B, C, H, W = 8, 512, 64, 64
N = H * W            # 4096 tokens
P = 128              # partitions
NCH = C // P         # 4 channel chunks
NOP = NCH // 2       # 2 channel-chunk pairs (DoubleRow)
BW = 512             # n-block width (= PSUM bank in fp32)
NB = N // BW         # 8 n-blocks
MT = N // P          # 32 m-tiles
MPAIR = MT // 2      # 16 m-tile pairs (DoubleRow)
G = 32               # groups
GS = C // G          # 16 channels per group
GPC = P // GS        # 8 groups per 128-channel chunk
NSG = 8              # GN stat slices per chunk (512 cols each)
EPS = 1e-6
EXP_SHIFT = 2.5      # exp(logit - shift); cancels in softmax normalization
NCORES = 8

# fp8 scale management (powers of 2; see module docstring)
QK_WBOOST = 32.0     # wq,wk stored * C^-0.25 * 32
QK_DSCALE = 0.125    # q epilogue: q' = psum/8 + qb'  (q' = q * C^-0.25 * 4)
EXP_SCALE = 0.0625   # logits' = 16 * logits
V_WBOOST = 8.0       # wv stored * 8
V_DSCALE = 0.125     # v epilogue: v' = psum/8 + vb  (v' = v)
P_WBOOST = 8.0       # wp stored * 8
ONES_VAL = 0.25      # ps_s = S/4 -> r = 4/S; hu = h_unnorm * 4/S
P_DSCALE = 1.0 / 32  # out = psum/32 + pb + x  (psum = 8*wp . 4*h_norm)

_EXEC = None


def _build_nc():
    import concourse.bacc as bacc
    import concourse.tile as tile
    from concourse import mybir

    f32 = mybir.dt.float32
    fp8 = mybir.dt.float8e4
    Alu = mybir.AluOpType
    Act = mybir.ActivationFunctionType
    DR = mybir.MatmulPerfMode.DoubleRow

    nc = bacc.Bacc(
        "TRN2", target_bir_lowering=False, debug=False, num_devices=NCORES
    )

    def din(name, shape, dt=f32):
        return nc.declare_dram_parameter(name, list(shape), dt, isOutput=False)

    x_d = din("x", [C, N])
    xb_d = din("xb", [C, N], mybir.dt.bfloat16)   # bf16 copy for GroupNorm
    wq_d = din("wq", [NOP, P, 2, C], fp8)   # pair layout, pre-scaled
    wk_d = din("wk", [NOP, P, 2, C], fp8)
    wv_d = din("wv", [NOP, P, 2, C], fp8)
    wp_d = din("wp", [NOP, P, 2, C], fp8)
    # [gamma, beta, qb, kb, pb] columns; pb has proj_w @ v_b folded in
    # (softmax sums to 1) -> one DMA for all per-channel vectors
    vecs_d = din("vecs", [C, 5])
    selsum_d = din("selsum", [P, GPC])
    selbc_d = din("selbc", [GPC, P])
    ones8_d = din("ones8", [P, 2 * P], fp8)   # DoubleRow 0.25s [P, 2, P]
    out_d = nc.declare_dram_parameter("out", [C, N], f32, isOutput=True)

    with tile.TileContext(nc) as tc:
        with (
            tc.tile_pool(name="consts", bufs=1) as consts,
            tc.tile_pool(name="qksb", bufs=1) as qkp,
            tc.tile_pool(name="vtsb", bufs=1) as vtp,
            tc.tile_pool(name="xsb", bufs=1) as xp,
        ):
            # ---- small consts first (GroupNorm needs them immediately);
            # batched into 4 DMAs (each dma_start costs ~650ns of sequencer
            # descriptor-gen, so fewer+bigger wins) ----
            vecs_sb = consts.tile([P, NCH, 5], f32, tag="vecs")
            nc.sync.dma_start(
                out=vecs_sb,
                in_=vecs_d[:, :].rearrange("(c p) v -> p c v", p=P),
            )
            gamma_sb = [vecs_sb[:, cc, 0:1] for cc in range(NCH)]
            beta_sb = [vecs_sb[:, cc, 1:2] for cc in range(NCH)]
            qb_sb = [vecs_sb[:, cc, 2:3] for cc in range(NCH)]
            kb_sb = [vecs_sb[:, cc, 3:4] for cc in range(NCH)]
            pb_sb = [vecs_sb[:, cc, 4:5] for cc in range(NCH)]
            selsum_sb = consts.tile([P, GPC], f32, tag="selsum")
            nc.sync.dma_start(out=selsum_sb, in_=selsum_d[:, :])
            selbc_sb = consts.tile([P, P], f32, tag="selbc")
            nc.sync.dma_start(out=selbc_sb[0:GPC, :], in_=selbc_d[:, :])
            ones8_sb = consts.tile([P, 2, P], fp8, tag="ones8")
            nc.sync.dma_start(
                out=ones8_sb,
                in_=ones8_d[:, :].rearrange("p (j q) -> p j q", j=2),
            )
            eps_sb = consts.tile([P, 1], f32, tag="eps")
            nc.vector.memset(eps_sb, EPS)
            negc_sb = consts.tile([P, 1], f32, tag="negc")
            nc.vector.memset(negc_sb, -EXP_SHIFT)

            # ---- x DMAs next (GN critical path); the GroupNorm input is
            # a host-cast bf16 copy (half the bytes on the critical DMA, 2x
            # bn_stats throughput; the residual re-reads the f32 x later,
            # fully overlapped). Half-chunk transfers for arrival pipelining.
            bf16 = mybir.dt.bfloat16
            x_sb = []
            for cc in range(NCH):
                xt = xp.tile([P, N], bf16, tag=f"x{cc}", name=f"x{cc}")
                for hh in range(2):
                    nsl = slice(hh * (N // 2), (hh + 1) * (N // 2))
                    nc.sync.dma_start(
                        out=xt[:, nsl], in_=xb_d[cc * P : (cc + 1) * P, nsl]
                    )
                x_sb.append(xt)

            # ---- weights on the gpsimd queue (concurrent with x; only
            # 1 MiB so they land early without delaying the x stream) ----
            def wpairs(d, tagp):
                ts = []
                for p in range(NOP):
                    t = consts.tile([P, 2, C], fp8, tag=f"{tagp}{p}",
                                    name=f"{tagp}{p}")
                    nc.gpsimd.dma_start(out=t, in_=d[p, :, :, :])
                    ts.append(t)
                return ts

            wq_sb = wpairs(wq_d, "wq")
            wk_sb = wpairs(wk_d, "wk")
            wv_sb = wpairs(wv_d, "wv")
            wp_sb = wpairs(wp_d, "wp")

            # q/k in DoubleRow pair layout: [P, 2, N], dim1 = pair member j,
            # channel chunk oc = 2*op + j; vT pairs [token-part, 2, C]
            q_sb = [qkp.tile([P, 2, N], fp8, tag=f"q{op}", name=f"q{op}")
                    for op in range(NOP)]
            k_sb = [qkp.tile([P, 2, N], fp8, tag=f"k{op}", name=f"k{op}")
                    for op in range(NOP)]
            vt_sb = [vtp.tile([P, 2, C], fp8, tag=f"vt{t}", name=f"vt{t}")
                     for t in range(MPAIR)]

            with tc.tile_pool(name="hsb", bufs=1) as hp:
                # h in fp8 pair layout for DR QKV matmuls
                h_sb = [hp.tile([P, 2, N], fp8, tag=f"h{p}", name=f"h{p}")
                        for p in range(NOP)]
                # ---- GroupNorm (per 128-channel chunk; groups don't cross) --
                with (
                    tc.tile_pool(name="gn", bufs=2) as gn,
                    tc.tile_pool(name="gnps", bufs=2, space="PSUM") as gnps,
                ):
                    for cc in range(NCH):
                        xt = x_sb[cc]
                        stats = gn.tile([P, NSG, 6], f32, tag="stats")
                        sw = N // NSG
                        for sg in range(NSG):
                            nc.vector.bn_stats(
                                out=stats[:, sg, :],
                                in_=xt[:, sg * sw : (sg + 1) * sw],
                            )
                        mv = gn.tile([P, 2], f32, tag="mv")
                        nc.vector.bn_aggr(out=mv, in_=stats)
                        # rhs2 = [mean_c, E[x^2]_c]
                        rhs2 = gn.tile([P, 2], f32, tag="rhs2")
                        nc.vector.tensor_copy(out=rhs2[:, 0:1], in_=mv[:, 0:1])
                        nc.vector.scalar_tensor_tensor(
                            out=rhs2[:, 1:2], in0=mv[:, 0:1], scalar=mv[:, 0:1],
                            in1=mv[:, 1:2], op0=Alu.mult, op1=Alu.add,
                        )
                        # group sums over the 16 channels of each group
                        g_ps = gnps.tile([P, 2], f32, tag="g_ps")
                        nc.tensor.matmul(
                            out=g_ps[0:GPC, :], lhsT=selsum_sb, rhs=rhs2,
                            start=True, stop=True,
                        )
                        gs_t = gn.tile([P, 2], f32, tag="gs")
                        nc.vector.tensor_scalar(
                            out=gs_t[0:GPC, :], in0=g_ps[0:GPC, :],
                            scalar1=1.0 / GS, scalar2=None, op0=Alu.mult,
                        )
                        mean2 = gn.tile([P, 1], f32, tag="mean2")
                        nc.vector.tensor_mul(mean2[0:GPC], gs_t[0:GPC, 0:1],
                                             gs_t[0:GPC, 0:1])
                        var = gn.tile([P, 1], f32, tag="var")
                        nc.vector.tensor_sub(var[0:GPC], gs_t[0:GPC, 1:2],
                                             mean2[0:GPC])
                        sq = gn.tile([P, 1], f32, tag="sq")
                        nc.scalar.activation(
                            out=sq[0:GPC], in_=var[0:GPC], func=Act.Sqrt,
                            bias=eps_sb[0:GPC], scale=1.0,
                        )
                        gmr = gn.tile([P, 2], f32, tag="gmr")
                        nc.vector.tensor_copy(out=gmr[0:GPC, 0:1],
                                              in_=gs_t[0:GPC, 0:1])
                        nc.vector.reciprocal(out=gmr[0:GPC, 1:2], in_=sq[0:GPC])
                        # broadcast (mean_g, rstd_g) back to channels
                        bc_ps = gnps.tile([P, 2], f32, tag="bc_ps")
                        nc.tensor.matmul(
                            out=bc_ps, lhsT=selbc_sb[0:GPC, :], rhs=gmr[0:GPC, :],
                            start=True, stop=True,
                        )
                        a_t = gn.tile([P, 1], f32, tag="a")
                        nc.vector.tensor_mul(a_t, bc_ps[:, 1:2], gamma_sb[cc])
                        na_t = gn.tile([P, 1], f32, tag="na")
                        nc.vector.tensor_scalar_mul(na_t, a_t, -1.0)
                        b_t = gn.tile([P, 1], f32, tag="b")
                        nc.vector.scalar_tensor_tensor(
                            out=b_t, in0=bc_ps[:, 0:1], scalar=na_t,
                            in1=beta_sb[cc], op0=Alu.mult, op1=Alu.add,
                        )
                        # h = x*a + b -> fp8 pair slice; last chunk on DVE
                        # (shorter critical path into the QKV matmuls)
                        if cc < NCH - 1:
                            nc.scalar.activation(
                                out=h_sb[cc // 2][:, cc % 2, :], in_=xt,
                                func=Act.Identity, scale=a_t, bias=b_t,
                            )
                        else:
                            nc.vector.tensor_scalar(
                                out=h_sb[cc // 2][:, cc % 2, :], in0=xt,
                                scalar1=a_t, scalar2=b_t,
                                op0=Alu.mult, op1=Alu.add,
                            )

                # pre-load the ACT EXP table after the last GroupNorm Sqrt
                # (otherwise a ~1.3us ACT_TABLE_LOAD lands right before the
                # first real EXP of the attention phase)
                expwarm_sb = consts.tile([P, 1], f32, tag="expwarm")
                nc.scalar.activation(
                    out=expwarm_sb, in_=eps_sb, func=Act.Exp, scale=1.0,
                )

                # ---- QKV projections (all fp8 DoubleRow); PSUM->fp8
                # epilogues alternate ACT/DVE so neither engine gates PE ----
                with tc.tile_pool(name="qkvps", bufs=4, space="PSUM") as qkvps:
                    for w_sb, b_sb, dst in ((wq_sb, qb_sb, q_sb),
                                            (wk_sb, kb_sb, k_sb)):
                        for oc in range(NCH):
                            osl = slice(oc * P, (oc + 1) * P)
                            for nt in range(NB):
                                nsl = slice(nt * BW, (nt + 1) * BW)
                                pt = qkvps.tile([P, BW], f32, tag="qkv")
                                for p in range(NOP):
                                    nc.tensor.matmul(
                                        out=pt,
                                        lhsT=w_sb[p][:, 0:2, osl],
                                        rhs=h_sb[p][:, 0:2, nsl],
                                        start=(p == 0), stop=(p == NOP - 1),
                                        perf_mode=DR,
                                    )
                                if nt % 2 == 0:
                                    nc.scalar.activation(
                                        out=dst[oc // 2][:, oc % 2, nsl],
                                        in_=pt, func=Act.Identity,
                                        scale=QK_DSCALE, bias=b_sb[oc],
                                    )
                                else:
                                    nc.vector.tensor_scalar(
                                        out=dst[oc // 2][:, oc % 2, nsl],
                                        in0=pt, scalar1=QK_DSCALE,
                                        scalar2=b_sb[oc],
                                        op0=Alu.mult, op1=Alu.add,
                                    )
                    # vT[m, o] = sum_c h[c, m] wv[c, o]; v_b is folded into
                    # pb on the host (softmax weights sum to 1)
                    for mt in range(MT):
                        msl = slice(mt * P, (mt + 1) * P)
                        pt = qkvps.tile([P, BW], f32, tag="qkv")
                        for p in range(NOP):
                            nc.tensor.matmul(
                                out=pt, lhsT=h_sb[p][:, 0:2, msl],
                                rhs=wv_sb[p], start=(p == 0),
                                stop=(p == NOP - 1), perf_mode=DR,
                            )
                        if mt % 2 == 0:
                            nc.scalar.activation(
                                out=vt_sb[mt // 2][:, mt % 2, :], in_=pt,
                                func=Act.Identity, scale=V_DSCALE,
                            )
                        else:
                            nc.vector.tensor_scalar(
                                out=vt_sb[mt // 2][:, mt % 2, :], in0=pt,
                                scalar1=V_DSCALE, scalar2=None, op0=Alu.mult,
                            )

            # ---- attention (fp8 DR) + interleaved delayed proj + residual --
            with (
                tc.tile_pool(name="eps_ps", bufs=2, space="PSUM") as e_ps,
                tc.tile_pool(name="s_ps", bufs=1, space="PSUM") as s_ps,
                tc.tile_pool(name="h_ps", bufs=1, space="PSUM") as h_ps,
                tc.tile_pool(name="p_ps", bufs=1, space="PSUM") as p_ps,
                tc.tile_pool(name="expt", bufs=8) as expt,
                tc.tile_pool(name="sumt", bufs=3) as sumt,
                tc.tile_pool(name="hup", bufs=2) as hup,
                tc.tile_pool(name="epil", bufs=2) as epil,
            ):
                def emit_proj_group(nbp, hu, oc2, last=False):
                    nsl = slice(nbp * BW, (nbp + 1) * BW)
                    osl = slice(oc2 * P, (oc2 + 1) * P)
                    pp = p_ps.tile([P, BW], f32, tag="p", name="pp")
                    for p in range(NOP):
                        nc.tensor.matmul(
                            out=pp, lhsT=wp_sb[p][:, 0:2, osl], rhs=hu[p],
                            start=(p == 0), stop=(p == NOP - 1), perf_mode=DR,
                        )
                    xt = epil.tile([P, BW], f32, tag="xt", name="xt")
                    nc.sync.dma_start(out=xt, in_=x_d[osl, nsl])
                    # out = pp/32 + pb + x; the last block's scale+bias goes
                    # on ACT (idle after the final EXP) to shorten the tail
                    t1 = epil.tile([P, BW], f32, tag="t1", name="t1")
                    if last:
                        nc.scalar.activation(
                            out=t1, in_=pp, func=Act.Identity,
                            scale=P_DSCALE, bias=pb_sb[oc2],
                        )
                    else:
                        nc.vector.tensor_scalar(
                            out=t1, in0=pp, scalar1=P_DSCALE,
                            scalar2=pb_sb[oc2], op0=Alu.mult, op1=Alu.add,
                        )
                    ot = epil.tile([P, BW], f32, tag="ot", name="ot")
                    nc.vector.tensor_tensor(out=ot, in0=t1, in1=xt, op=Alu.add)
                    nc.sync.dma_start(out=out_d[osl, nsl], in_=ot)

                def emit_av(pr, et, ph):
                    """attn.v matmuls for one m-pair (consume et)."""
                    for oc in range(NCH):
                        nc.tensor.matmul(
                            out=ph[oc],
                            lhsT=vt_sb[pr][:, 0:2, oc * P : (oc + 1) * P],
                            rhs=et,
                            start=(pr == 0), stop=(pr == MPAIR - 1),
                            perf_mode=DR,
                        )

                def emit_ones(g, sg, ps_s, ng):
                    nc.tensor.matmul(
                        out=ps_s, lhsT=ones8_sb, rhs=sg,
                        start=(g == 0), stop=(g == ng - 1),
                        perf_mode=DR,
                    )

                pending = None
                for nb in range(NB):
                    nsl = slice(nb * BW, (nb + 1) * BW)
                    ps_s = s_ps.tile([P, BW], f32, tag="s", name="ps_s")
                    ph = [h_ps.tile([P, BW], f32, tag=f"h{oc}", name=f"hps{oc}")
                          for oc in range(NCH)]
                    # software pipeline: AV runs one m-pair behind the E
                    # matmuls so PE never waits on the EXP latency; the
                    # softmax denominator sums groups of 4 exp tiles on the
                    # (otherwise idle) GpSimd engine so only one ones-matmul
                    # per group hits the PE
                    ets, sgs, roots = [], [], []
                    for pr in range(MPAIR):
                        et = expt.tile([P, 2, BW], fp8, tag="et", name="et")
                        for j in range(2):
                            mt = 2 * pr + j
                            msl = slice(mt * P, (mt + 1) * P)
                            pe = e_ps.tile([P, BW], f32, tag="e", name="pe")
                            for op in range(NOP):
                                nc.tensor.matmul(
                                    out=pe, lhsT=k_sb[op][:, 0:2, msl],
                                    rhs=q_sb[op][:, 0:2, nsl],
                                    start=(op == 0), stop=(op == NOP - 1),
                                    perf_mode=DR,
                                )
                            nc.scalar.activation(
                                out=et[:, j, :], in_=pe, func=Act.Exp,
                                bias=negc_sb, scale=EXP_SCALE,
                            )
                        ets.append(et)
                        # denominator pre-sums: prs 0-11 in three 4-way
                        # groups (gpsimd pair-sums + DVE root), prs 12-13 a
                        # single 2-way, prs 14-15 consumed directly by the
                        # ones-matmul (keeps the block-boundary chain free
                        # of add latency); values <~100 << fp8 max 240
                        if pr % 2 == 1 and pr <= 13:
                            sg = sumt.tile([P, 2, BW], fp8, tag="sg",
                                           name="sg")
                            nc.gpsimd.tensor_tensor(
                                out=sg, in0=ets[pr - 1], in1=ets[pr],
                                op=Alu.add,
                            )
                            sgs.append(sg)
                            if pr % 4 == 3:
                                rt = sumt.tile([P, 2, BW], fp8, tag="rt",
                                               name="rt")
                                nc.vector.tensor_tensor(
                                    out=rt, in0=sgs[-2], in1=sgs[-1],
                                    op=Alu.add,
                                )
                                roots.append(rt)
                        if pr > 0:
                            emit_av(pr - 1, ets[pr - 1], ph)
                            # spread prev block's proj through the m-loop
                            if pending is not None and pr % 4 == 0:
                                emit_proj_group(pending[0], pending[1],
                                                pr // 4 - 1)
                            # root ones-matmuls trail their sums by 4 m-pairs
                            if pr % 4 == 3 and 7 <= pr <= 15:
                                emit_ones(pr // 4 - 1, roots[pr // 4 - 1],
                                          ps_s, 6)
                    emit_av(MPAIR - 1, ets[MPAIR - 1], ph)
                    if pending is not None:
                        emit_proj_group(pending[0], pending[1], NCH - 1)
                    emit_ones(3, sgs[6], ps_s, 6)
                    emit_ones(4, ets[14], ps_s, 6)
                    emit_ones(5, ets[15], ps_s, 6)
                    # r = 4/S (ones are 0.25s); normalize into fp8 pairs on
                    # DVE (keeps ACT pure-EXP)
                    r_t = epil.tile([P, BW], f32, tag="r", name="r_t")
                    nc.vector.reciprocal_approx_fast(out=r_t, in_=ps_s)
                    hu = [hup.tile([P, 2, BW], fp8, tag=f"hu{p}",
                                   name=f"hu{p}") for p in range(NOP)]
                    for oc in range(NCH):
                        nc.vector.tensor_tensor(
                            out=hu[oc // 2][:, oc % 2, :], in0=ph[oc],
                            in1=r_t, op=Alu.mult,
                        )
                    pending = (nb, hu)
                for oc2 in range(NCH):
                    emit_proj_group(pending[0], pending[1], oc2, last=True)

    nc.compile()
    return nc


def _build_exec():
    import jax
    from jax.experimental.shard_map import shard_map
    from jax.sharding import Mesh, PartitionSpec

    from concourse import bass2jax, mybir

    nc = _build_nc()
    bass2jax.install_neuronx_cc_hook()

    partition_name = (
        nc.partition_id_tensor.name if nc.partition_id_tensor else None
    )
    in_names, out_names, out_avals = [], [], []
    for alloc in nc.m.functions[0].allocations:
        if not isinstance(alloc, mybir.MemoryLocationSet):
            continue
        name = alloc.memorylocations[0].name
        if alloc.kind == "ExternalInput":
            if name != partition_name:
                in_names.append(name)
        elif alloc.kind == "ExternalOutput":
            out_names.append(name)
            out_avals.append(
                jax.core.ShapedArray(
                    tuple(alloc.tensor_shape), mybir.dt.np(alloc.dtype)
                )
            )
    n_params = len(in_names)
    all_in = tuple(in_names + out_names)
    if partition_name is not None:
        all_in = all_in + (partition_name,)
    donate = tuple(range(n_params, n_params + len(out_names)))

    def _body(*args):
        operands = list(args)
        if partition_name is not None:
            operands.append(bass2jax.partition_id_tensor())
        outs = bass2jax._bass_exec_p.bind(
            *operands,
            out_avals=tuple(out_avals),
            in_names=all_in,
            out_names=tuple(out_names),
            lowering_input_output_aliases=(),
            sim_require_finite=True,
            sim_require_nnan=True,
            nc=nc,
        )
        return tuple(outs)

    devices = jax.devices()[:NCORES]
    mesh = Mesh(np.asarray(devices), ("core",))
    in_specs = (PartitionSpec("core"),) * (n_params + len(out_names))
    out_specs = (PartitionSpec("core"),) * len(out_names)
    sharded = jax.jit(
        shard_map(
            _body, mesh=mesh, in_specs=in_specs, out_specs=out_specs,
            check_rep=False,
        ),
        donate_argnums=donate,
        keep_unused=True,
    )
    return sharded, in_names, out_names, out_avals, nc


def _get_exec():
    global _EXEC
    if _EXEC is None:
        _EXEC = _build_exec()
    return _EXEC


def _selsum():
    s = np.zeros((P, GPC), np.float32)
    s[np.arange(P), np.arange(P) // GS] = 1.0
    return s


def _pair_fp8(w, boost):
    """[C_out, C_in] torch-style weight -> fp8 DR pair layout [NOP,P,2,C]."""
    wt = np.ascontiguousarray(np.asarray(w, np.float32).T) * boost
    return np.ascontiguousarray(
        wt.reshape(NOP, 2, P, C).transpose(0, 2, 1, 3)
    ).astype(FP8)


def make_concat_inputs(inputs):
    """Host-side prep: per-core shards concatenated on axis 0 (shard_map)."""
    x = np.asarray(inputs["x"], np.float32).reshape(B, C, N)
    sqs = np.float32(C ** -0.25)

    shared = {
        "wq": _pair_fp8(inputs["q_w"], sqs * QK_WBOOST),
        "wk": _pair_fp8(inputs["k_w"], sqs * QK_WBOOST),
        "wv": _pair_fp8(inputs["v_w"], V_WBOOST),
        "wp": _pair_fp8(inputs["proj_w"], P_WBOOST),
        # [gamma, beta, qb', kb', pb'] columns; pb' folds in proj_w @ v_b
        # (softmax weights sum to 1, so v_b is a constant proj-side shift)
        "vecs": np.ascontiguousarray(np.stack([
            np.asarray(inputs["gamma"], np.float32),
            np.asarray(inputs["beta"], np.float32),
            np.asarray(inputs["q_b"], np.float32)
            * (sqs * QK_WBOOST * QK_DSCALE),
            np.asarray(inputs["k_b"], np.float32)
            * (sqs * QK_WBOOST * QK_DSCALE),
            np.asarray(inputs["proj_b"], np.float32)
            + np.asarray(inputs["proj_w"], np.float32)
            @ np.asarray(inputs["v_b"], np.float32),
        ], axis=1)),
        "selsum": _selsum(),
        "selbc": np.ascontiguousarray(_selsum().T),
        "ones8": np.full((P, 2 * P), ONES_VAL, FP8),
    }
    per_core = [dict(shared, x=np.ascontiguousarray(x[c]),
                     xb=np.ascontiguousarray(x[c].astype(BF16)))
                for c in range(NCORES)]

    sharded, in_names, out_names, out_avals, _ = _get_exec()
    concat_in = [
        np.concatenate([per_core[c][nm] for c in range(NCORES)], axis=0)
        for nm in in_names
    ]
    return concat_in, out_avals


def run_concat(concat_in, out_avals):
    sharded = _get_exec()[0]
    concat_zeros = [
        np.zeros((NCORES * av.shape[0], *av.shape[1:]), av.dtype)
        for av in out_avals
    ]
    outs = sharded(*concat_in, *concat_zeros)
    return outs


def kernel(**inputs):
    concat_in, out_avals = make_concat_inputs(inputs)
    outs = run_concat(concat_in, out_avals)
    o = np.asarray(outs[0]).reshape(NCORES, C, N)
    return np.ascontiguousarray(o.reshape(B, C, H, W), dtype=np.float32)


# revision 27
# speedup vs baseline: 1.2491x; 1.0034x over previous
"""Trainium2 Bass kernel for nn_AttentionBlock (GroupNorm -> QKV 1x1 -> softmax
attention over 4096 tokens -> proj + residual).

Sharding: pure data-parallel over batch B=8 across the 8 NeuronCores (one
batch element per core); attention is per-batch-element so no collectives.

Per-core layout (C=512 channels, N=4096 tokens), all matmuls fp8e4 DoubleRow:
  - x loaded [channel-part, token] as 4 chunks of [128, 4096], DMA'd per
    512-col slice FIRST (before weights) so GroupNorm stats pipeline behind
    the DMA instead of serializing after it
  - GroupNorm fp32 (bn_stats per slice; group reduce/broadcast via tiny
    matmuls); h stored fp8 in DoubleRow pair layout [128, 2, 4096]
  - weights pre-scaled by powers of 2 so every fp8 tensor sits in e4m3's
    sweet spot; compensation folded into activation scales:
      wq,wk *= C^-0.25 * 32 (q,k stored as q*C^-0.25*4 -> logits' = 16*logit,
      EXP uses scale=1/16); wv *= 8 (v epilogue scale 1/8); wp *= 8
  - logits computed transposed: E^T[m, n] = sum_o k[o,m] q[o,n]; softmax via
    exp(logit - 2.5) in fp8 (shift cancels in normalization); denominator
    via a 0.25-valued ones-matmul so reciprocal gives 4/S directly
  - h_attn_unnorm[o, n] = sum_m vT[m, o] expT[m, n]; normalized by 4/S on
    the Vector engine into fp8 pairs (keeps Scalar engine pure-EXP during
    attention -> no ACT table thrash), then proj as fp8 DR one n-block
    behind, its 4 output groups interleaved into the next block's m-loop
  - final: out = pp/32 + pb + x (residual re-DMA'd, overlapped)

Self-contained: hardcodes shapes; builds + compiles the Bass graph once and
caches a persistent jitted shard_map executor over the 8 axon NeuronCores.
"""

import os
import sys

sys.path.insert(0, "/opt/trn_rl_repo")
os.environ.setdefault("MYCRO_LOCAL_CACHE", "1")

import numpy as np
import ml_dtypes

BF16 = ml_dtypes.bfloat16
FP8 = ml_dtypes.float8_e4m3

# Problem constants (hardcoded; kernel.py must not read spec/reference files)
B, C, H, W = 8, 512, 64, 64
N = H * W            # 4096 tokens
P = 128              # partitions
NCH = C // P         # 4 channel chunks
NOP = NCH // 2       # 2 channel-chunk pairs (DoubleRow)
BW = 512             # n-block width (= PSUM bank in fp32)
NB = N // BW         # 8 n-blocks
MT = N // P          # 32 m-tiles
MPAIR = MT // 2      # 16 m-tile pairs (DoubleRow)
G = 32               # groups
GS = C // G          # 16 channels per group
GPC = P // GS        # 8 groups per 128-channel chunk
NSG = 8              # GN stat slices per chunk (512 cols each)
EPS = 1e-6
EXP_SHIFT = 2.5      # exp(logit - shift); cancels in softmax normalization
NCORES = 8

# fp8 scale management (powers of 2; see module docstring)
QK_WBOOST = 32.0     # wq,wk stored * C^-0.25 * 32
QK_DSCALE = 0.125    # q epilogue: q' = psum/8 + qb'  (q' = q * C^-0.25 * 4)
EXP_SCALE = 0.0625   # logits' = 16 * logits
V_WBOOST = 8.0       # wv stored * 8
V_DSCALE = 0.125     # v epilogue: v' = psum/8 + vb  (v' = v)
P_WBOOST = 8.0       # wp stored * 8
ONES_VAL = 0.25      # ps_s = S/4 -> r = 4/S; hu = h_unnorm * 4/S
P_DSCALE = 1.0 / 32  # out = psum/32 + pb + x  (psum = 8*wp . 4*h_norm)

_EXEC = None


def _build_nc():
    import concourse.bacc as bacc
    import concourse.tile as tile
    from concourse import mybir

    f32 = mybir.dt.float32
    fp8 = mybir.dt.float8e4
    Alu = mybir.AluOpType
    Act = mybir.ActivationFunctionType
    DR = mybir.MatmulPerfMode.DoubleRow

    nc = bacc.Bacc(
        "TRN2", target_bir_lowering=False, debug=False, num_devices=NCORES
    )

    def din(name, shape, dt=f32):
        return nc.declare_dram_parameter(name, list(shape), dt, isOutput=False)

    x_d = din("x", [C, N])
    xb_d = din("xb", [C, N], mybir.dt.bfloat16)   # bf16 copy for GroupNorm
    wq_d = din("wq", [NOP, P, 2, C], fp8)   # pair layout, pre-scaled
    wk_d = din("wk", [NOP, P, 2, C], fp8)
    wv_d = din("wv", [NOP, P, 2, C], fp8)
    wp_d = din("wp", [NOP, P, 2, C], fp8)
    # [gamma, beta, qb, kb, pb] columns; pb has proj_w @ v_b folded in
    # (softmax sums to 1) -> one DMA for all per-channel vectors
    vecs_d = din("vecs", [C, 5])
    selsum_d = din("selsum", [P, GPC])
    selbc_d = din("selbc", [GPC, P])
    ones8_d = din("ones8", [P, 2 * P], fp8)   # DoubleRow 0.25s [P, 2, P]
    out_d = nc.declare_dram_parameter("out", [C, N], f32, isOutput=True)

    with tile.TileContext(nc) as tc:
        with (
            tc.tile_pool(name="consts", bufs=1) as consts,
            tc.tile_pool(name="qksb", bufs=1) as qkp,
            tc.tile_pool(name="vtsb", bufs=1) as vtp,
            tc.tile_pool(name="xsb", bufs=1) as xp,
        ):
            # ---- small consts first (GroupNorm needs them immediately);
            # batched into 4 DMAs (each dma_start costs ~650ns of sequencer
            # descriptor-gen, so fewer+bigger wins) ----
            vecs_sb = consts.tile([P, NCH, 5], f32, tag="vecs")
            nc.sync.dma_start(
                out=vecs_sb,
                in_=vecs_d[:, :].rearrange("(c p) v -> p c v", p=P),
            )
            gamma_sb = [vecs_sb[:, cc, 0:1] for cc in range(NCH)]
            beta_sb = [vecs_sb[:, cc, 1:2] for cc in range(NCH)]
            qb_sb = [vecs_sb[:, cc, 2:3] for cc in range(NCH)]
            kb_sb = [vecs_sb[:, cc, 3:4] for cc in range(NCH)]
            pb_sb = [vecs_sb[:, cc, 4:5] for cc in range(NCH)]
            selsum_sb = consts.tile([P, GPC], f32, tag="selsum")
            nc.sync.dma_start(out=selsum_sb, in_=selsum_d[:, :])
            selbc_sb = consts.tile([P, P], f32, tag="selbc")
            nc.sync.dma_start(out=selbc_sb[0:GPC, :], in_=selbc_d[:, :])
            ones8_sb = consts.tile([P, 2, P], fp8, tag="ones8")
            nc.sync.dma_start(
                out=ones8_sb,
                in_=ones8_d[:, :].rearrange("p (j q) -> p j q", j=2),
            )
            eps_sb = consts.tile([P, 1], f32, tag="eps")
            nc.vector.memset(eps_sb, EPS)
            negc_sb = consts.tile([P, 1], f32, tag="negc")
            nc.vector.memset(negc_sb, -EXP_SHIFT)

            # ---- x DMAs next (GN critical path); the GroupNorm input is
            # a host-cast bf16 copy (half the bytes on the critical DMA, 2x
            # bn_stats throughput; the residual re-reads the f32 x later,
            # fully overlapped). Half-chunk transfers for arrival pipelining.
            bf16 = mybir.dt.bfloat16
            x_sb = []
            for cc in range(NCH):
                xt = xp.tile([P, N], bf16, tag=f"x{cc}", name=f"x{cc}")
                nc.sync.dma_start(out=xt, in_=xb_d[cc * P : (cc + 1) * P, :])
                x_sb.append(xt)

            # ---- weights on the gpsimd queue (concurrent with x; only
            # 1 MiB so they land early without delaying the x stream) ----
            def wpairs(d, tagp):
                ts = []
                for p in range(NOP):
                    t = consts.tile([P, 2, C], fp8, tag=f"{tagp}{p}",
                                    name=f"{tagp}{p}")
                    nc.gpsimd.dma_start(out=t, in_=d[p, :, :, :])
                    ts.append(t)
                return ts

            wq_sb = wpairs(wq_d, "wq")
            wk_sb = wpairs(wk_d, "wk")
            wv_sb = wpairs(wv_d, "wv")
            wp_sb = wpairs(wp_d, "wp")

            # q/k in DoubleRow pair layout: [P, 2, N], dim1 = pair member j,
            # channel chunk oc = 2*op + j; vT pairs [token-part, 2, C]
            q_sb = [qkp.tile([P, 2, N], fp8, tag=f"q{op}", name=f"q{op}")
                    for op in range(NOP)]
            k_sb = [qkp.tile([P, 2, N], fp8, tag=f"k{op}", name=f"k{op}")
                    for op in range(NOP)]
            vt_sb = [vtp.tile([P, 2, C], fp8, tag=f"vt{t}", name=f"vt{t}")
                     for t in range(MPAIR)]

            with tc.tile_pool(name="hsb", bufs=1) as hp:
                # h in fp8 pair layout for DR QKV matmuls
                h_sb = [hp.tile([P, 2, N], fp8, tag=f"h{p}", name=f"h{p}")
                        for p in range(NOP)]
                # ---- GroupNorm (per 128-channel chunk; groups don't cross) --
                with (
                    tc.tile_pool(name="gn", bufs=2) as gn,
                    tc.tile_pool(name="gnps", bufs=2, space="PSUM") as gnps,
                ):
                    for cc in range(NCH):
                        xt = x_sb[cc]
                        stats = gn.tile([P, NSG, 6], f32, tag="stats")
                        sw = N // NSG
                        for sg in range(NSG):
                            nc.vector.bn_stats(
                                out=stats[:, sg, :],
                                in_=xt[:, sg * sw : (sg + 1) * sw],
                            )
                        mv = gn.tile([P, 2], f32, tag="mv")
                        nc.vector.bn_aggr(out=mv, in_=stats)
                        # rhs2 = [mean_c, E[x^2]_c]
                        rhs2 = gn.tile([P, 2], f32, tag="rhs2")
                        nc.vector.tensor_copy(out=rhs2[:, 0:1], in_=mv[:, 0:1])
                        nc.vector.scalar_tensor_tensor(
                            out=rhs2[:, 1:2], in0=mv[:, 0:1], scalar=mv[:, 0:1],
                            in1=mv[:, 1:2], op0=Alu.mult, op1=Alu.add,
                        )
                        # group sums over the 16 channels of each group
                        g_ps = gnps.tile([P, 2], f32, tag="g_ps")
                        nc.tensor.matmul(
                            out=g_ps[0:GPC, :], lhsT=selsum_sb, rhs=rhs2,
                            start=True, stop=True,
                        )
                        gs_t = gn.tile([P, 2], f32, tag="gs")
                        nc.vector.tensor_scalar(
                            out=gs_t[0:GPC, :], in0=g_ps[0:GPC, :],
                            scalar1=1.0 / GS, scalar2=None, op0=Alu.mult,
                        )
                        mean2 = gn.tile([P, 1], f32, tag="mean2")
                        nc.vector.tensor_mul(mean2[0:GPC], gs_t[0:GPC, 0:1],
                                             gs_t[0:GPC, 0:1])
                        var = gn.tile([P, 1], f32, tag="var")
                        nc.vector.tensor_sub(var[0:GPC], gs_t[0:GPC, 1:2],
                                             mean2[0:GPC])
                        sq = gn.tile([P, 1], f32, tag="sq")
                        nc.scalar.activation(
                            out=sq[0:GPC], in_=var[0:GPC], func=Act.Sqrt,
                            bias=eps_sb[0:GPC], scale=1.0,
                        )
                        gmr = gn.tile([P, 2], f32, tag="gmr")
                        nc.vector.tensor_copy(out=gmr[0:GPC, 0:1],
                                              in_=gs_t[0:GPC, 0:1])
                        nc.vector.reciprocal(out=gmr[0:GPC, 1:2], in_=sq[0:GPC])
                        # broadcast (mean_g, rstd_g) back to channels
                        bc_ps = gnps.tile([P, 2], f32, tag="bc_ps")
                        nc.tensor.matmul(
                            out=bc_ps, lhsT=selbc_sb[0:GPC, :], rhs=gmr[0:GPC, :],
                            start=True, stop=True,
                        )
                        a_t = gn.tile([P, 1], f32, tag="a")
                        nc.vector.tensor_mul(a_t, bc_ps[:, 1:2], gamma_sb[cc])
                        na_t = gn.tile([P, 1], f32, tag="na")
                        nc.vector.tensor_scalar_mul(na_t, a_t, -1.0)
                        b_t = gn.tile([P, 1], f32, tag="b")
                        nc.vector.scalar_tensor_tensor(
                            out=b_t, in0=bc_ps[:, 0:1], scalar=na_t,
                            in1=beta_sb[cc], op0=Alu.mult, op1=Alu.add,
                        )
                        if cc == NCH - 1:
                            last_bt = b_t
                        # h = x*a + b -> fp8 pair slice; last chunk on DVE
                        # (shorter critical path into the QKV matmuls)
                        if cc < NCH - 1:
                            nc.scalar.activation(
                                out=h_sb[cc // 2][:, cc % 2, :], in_=xt,
                                func=Act.Identity, scale=a_t, bias=b_t,
                            )
                        else:
                            nc.vector.tensor_scalar(
                                out=h_sb[cc // 2][:, cc % 2, :], in0=xt,
                                scalar1=a_t, scalar2=b_t,
                                op0=Alu.mult, op1=Alu.add,
                            )

                # pre-load the ACT EXP table after the last GroupNorm Sqrt
                # (otherwise a ~1.3us ACT_TABLE_LOAD lands right before the
                # first real EXP of the attention phase)
                expwarm_sb = consts.tile([P, 1], f32, tag="expwarm")
                nc.scalar.activation(
                    out=expwarm_sb, in_=last_bt, func=Act.Exp, scale=0.0,
                )

                # ---- QKV projections (all fp8 DoubleRow); PSUM->fp8
                # epilogues alternate ACT/DVE so neither engine gates PE ----
                with tc.tile_pool(name="qkvps", bufs=4, space="PSUM") as qkvps:
                    for w_sb, b_sb, dst in ((wq_sb, qb_sb, q_sb),
                                            (wk_sb, kb_sb, k_sb)):
                        for oc in range(NCH):
                            osl = slice(oc * P, (oc + 1) * P)
                            for nt in range(NB):
                                nsl = slice(nt * BW, (nt + 1) * BW)
                                pt = qkvps.tile([P, BW], f32, tag="qkv")
                                for p in range(NOP):
                                    nc.tensor.matmul(
                                        out=pt,
                                        lhsT=w_sb[p][:, 0:2, osl],
                                        rhs=h_sb[p][:, 0:2, nsl],
                                        start=(p == 0), stop=(p == NOP - 1),
                                        perf_mode=DR,
                                    )
                                if nt % 2 == 0:
                                    nc.scalar.activation(
                                        out=dst[oc // 2][:, oc % 2, nsl],
                                        in_=pt, func=Act.Identity,
                                        scale=QK_DSCALE, bias=b_sb[oc],
                                    )
                                else:
                                    nc.vector.tensor_scalar(
                                        out=dst[oc // 2][:, oc % 2, nsl],
                                        in0=pt, scalar1=QK_DSCALE,
                                        scalar2=b_sb[oc],
                                        op0=Alu.mult, op1=Alu.add,
                                    )
                    # vT[m, o] = sum_c h[c, m] wv[c, o]; v_b is folded into
                    # pb on the host (softmax weights sum to 1)
                    for mt in range(MT):
                        msl = slice(mt * P, (mt + 1) * P)
                        pt = qkvps.tile([P, BW], f32, tag="qkv")
                        for p in range(NOP):
                            nc.tensor.matmul(
                                out=pt, lhsT=h_sb[p][:, 0:2, msl],
                                rhs=wv_sb[p], start=(p == 0),
                                stop=(p == NOP - 1), perf_mode=DR,
                            )
                        if mt % 2 == 0:
                            nc.scalar.activation(
                                out=vt_sb[mt // 2][:, mt % 2, :], in_=pt,
                                func=Act.Identity, scale=V_DSCALE,
                            )
                        else:
                            nc.vector.tensor_scalar(
                                out=vt_sb[mt // 2][:, mt % 2, :], in0=pt,
                                scalar1=V_DSCALE, scalar2=None, op0=Alu.mult,
                            )

            # ---- attention (fp8 DR) + interleaved delayed proj + residual --
            with (
                tc.tile_pool(name="eps_ps", bufs=2, space="PSUM") as e_ps,
                tc.tile_pool(name="s_ps", bufs=1, space="PSUM") as s_ps,
                tc.tile_pool(name="h_ps", bufs=1, space="PSUM") as h_ps,
                tc.tile_pool(name="p_ps", bufs=1, space="PSUM") as p_ps,
                tc.tile_pool(name="expt", bufs=8) as expt,
                tc.tile_pool(name="sumt", bufs=3) as sumt,
                tc.tile_pool(name="hup", bufs=2) as hup,
                tc.tile_pool(name="epil", bufs=2) as epil,
            ):
                def emit_proj_group(nbp, hu, oc2, last=False):
                    nsl = slice(nbp * BW, (nbp + 1) * BW)
                    osl = slice(oc2 * P, (oc2 + 1) * P)
                    pp = p_ps.tile([P, BW], f32, tag="p", name="pp")
                    for p in range(NOP):
                        nc.tensor.matmul(
                            out=pp, lhsT=wp_sb[p][:, 0:2, osl], rhs=hu[p],
                            start=(p == 0), stop=(p == NOP - 1), perf_mode=DR,
                        )
                    xt = epil.tile([P, BW], f32, tag="xt", name="xt")
                    nc.sync.dma_start(out=xt, in_=x_d[osl, nsl])
                    # out = pp/32 + pb + x; the last block's scale+bias goes
                    # on ACT (idle after the final EXP) to shorten the tail
                    t1 = epil.tile([P, BW], f32, tag="t1", name="t1")
                    if last:
                        nc.scalar.activation(
                            out=t1, in_=pp, func=Act.Identity,
                            scale=P_DSCALE, bias=pb_sb[oc2],
                        )
                    else:
                        nc.vector.tensor_scalar(
                            out=t1, in0=pp, scalar1=P_DSCALE,
                            scalar2=pb_sb[oc2], op0=Alu.mult, op1=Alu.add,
                        )
                    ot = epil.tile([P, BW], f32, tag="ot", name="ot")
                    nc.vector.tensor_tensor(out=ot, in0=t1, in1=xt, op=Alu.add)
                    nc.sync.dma_start(out=out_d[osl, nsl], in_=ot)

                def emit_av(pr, et, ph):
                    """attn.v matmuls for one m-pair (consume et)."""
                    for oc in range(NCH):
                        nc.tensor.matmul(
                            out=ph[oc],
                            lhsT=vt_sb[pr][:, 0:2, oc * P : (oc + 1) * P],
                            rhs=et,
                            start=(pr == 0), stop=(pr == MPAIR - 1),
                            perf_mode=DR,
                        )

                def emit_ones(g, sg, ps_s, ng):
                    nc.tensor.matmul(
                        out=ps_s, lhsT=ones8_sb, rhs=sg,
                        start=(g == 0), stop=(g == ng - 1),
                        perf_mode=DR,
                    )

                pending = None
                for nb in range(NB):
                    nsl = slice(nb * BW, (nb + 1) * BW)
                    ps_s = s_ps.tile([P, BW], f32, tag="s", name="ps_s")
                    ph = [h_ps.tile([P, BW], f32, tag=f"h{oc}", name=f"hps{oc}")
                          for oc in range(NCH)]
                    # software pipeline: AV runs one m-pair behind the E
                    # matmuls so PE never waits on the EXP latency; the
                    # softmax denominator sums groups of 4 exp tiles on the
                    # (otherwise idle) GpSimd engine so only one ones-matmul
                    # per group hits the PE
                    ets, sgs, roots = [], [], []
                    for pr in range(MPAIR):
                        et = expt.tile([P, 2, BW], fp8, tag="et", name="et")
                        for j in range(2):
                            mt = 2 * pr + j
                            msl = slice(mt * P, (mt + 1) * P)
                            pe = e_ps.tile([P, BW], f32, tag="e", name="pe")
                            for op in range(NOP):
                                nc.tensor.matmul(
                                    out=pe, lhsT=k_sb[op][:, 0:2, msl],
                                    rhs=q_sb[op][:, 0:2, nsl],
                                    start=(op == 0), stop=(op == NOP - 1),
                                    perf_mode=DR,
                                )
                            nc.scalar.activation(
                                out=et[:, j, :], in_=pe, func=Act.Exp,
                                bias=negc_sb, scale=EXP_SCALE,
                            )
                        ets.append(et)
                        # denominator pre-sums: prs 0-11 in three 4-way
                        # groups (gpsimd pair-sums + DVE root), prs 12-13 a
                        # single 2-way, prs 14-15 consumed directly by the
                        # ones-matmul (keeps the block-boundary chain free
                        # of add latency); values <~100 << fp8 max 240
                        if pr % 2 == 1 and pr <= 13:
                            sg = sumt.tile([P, 2, BW], fp8, tag="sg",
                                           name="sg")
                            nc.gpsimd.tensor_tensor(
                                out=sg, in0=ets[pr - 1], in1=ets[pr],
                                op=Alu.add,
                            )
                            sgs.append(sg)
                            if pr % 4 == 3:
                                rt = sumt.tile([P, 2, BW], fp8, tag="rt",
                                               name="rt")
                                nc.vector.tensor_tensor(
                                    out=rt, in0=sgs[-2], in1=sgs[-1],
                                    op=Alu.add,
                                )
                                roots.append(rt)
                        if pr > 0:
                            emit_av(pr - 1, ets[pr - 1], ph)
                            # spread prev block's proj through the m-loop
                            if pending is not None and pr % 4 == 0:
                                emit_proj_group(pending[0], pending[1],
                                                pr // 4 - 1)
                            # root ones-matmuls trail their sums by 4 m-pairs
                            if pr % 4 == 3 and 7 <= pr <= 15:
                                emit_ones(pr // 4 - 1, roots[pr // 4 - 1],
                                          ps_s, 6)
                    emit_av(MPAIR - 1, ets[MPAIR - 1], ph)
                    if pending is not None:
                        emit_proj_group(pending[0], pending[1], NCH - 1)
                    emit_ones(3, sgs[6], ps_s, 6)
                    emit_ones(4, ets[14], ps_s, 6)
                    emit_ones(5, ets[15], ps_s, 6)
                    # r = 4/S (ones are 0.25s); normalize into fp8 pairs on
                    # DVE (keeps ACT pure-EXP)
                    r_t = epil.tile([P, BW], f32, tag="r", name="r_t")
                    nc.vector.reciprocal_approx_fast(out=r_t, in_=ps_s)
                    hu = [hup.tile([P, 2, BW], fp8, tag=f"hu{p}",
                                   name=f"hu{p}") for p in range(NOP)]
                    for oc in range(NCH):
                        nc.vector.tensor_tensor(
                            out=hu[oc // 2][:, oc % 2, :], in0=ph[oc],
                            in1=r_t, op=Alu.mult,
                        )
                    pending = (nb, hu)
                for oc2 in range(NCH):
                    emit_proj_group(pending[0], pending[1], oc2, last=True)

    nc.compile()
    return nc


def _build_exec():
    import jax
    from jax.experimental.shard_map import shard_map
    from jax.sharding import Mesh, PartitionSpec

    from concourse import bass2jax, mybir

    nc = _build_nc()
    bass2jax.install_neuronx_cc_hook()

    partition_name = (
        nc.partition_id_tensor.name if nc.partition_id_tensor else None
    )
    in_names, out_names, out_avals = [], [], []
    for alloc in nc.m.functions[0].allocations:
        if not isinstance(alloc, mybir.MemoryLocationSet):
            continue
        name = alloc.memorylocations[0].name
        if alloc.kind == "ExternalInput":
            if name != partition_name:
                in_names.append(name)
        elif alloc.kind == "ExternalOutput":
            out_names.append(name)
            out_avals.append(
                jax.core.ShapedArray(
                    tuple(alloc.tensor_shape), mybir.dt.np(alloc.dtype)
                )
            )
    n_params = len(in_names)
    all_in = tuple(in_names + out_names)
    if partition_name is not None:
        all_in = all_in + (partition_name,)
    donate = tuple(range(n_params, n_params + len(out_names)))

    def _body(*args):
        operands = list(args)
        if partition_name is not None:
            operands.append(bass2jax.partition_id_tensor())
        outs = bass2jax._bass_exec_p.bind(
            *operands,
            out_avals=tuple(out_avals),
            in_names=all_in,
            out_names=tuple(out_names),
            lowering_input_output_aliases=(),
            sim_require_finite=True,
            sim_require_nnan=True,
            nc=nc,
        )
        return tuple(outs)

    devices = jax.devices()[:NCORES]
    mesh = Mesh(np.asarray(devices), ("core",))
    in_specs = (PartitionSpec("core"),) * (n_params + len(out_names))
    out_specs = (PartitionSpec("core"),) * len(out_names)
    sharded = jax.jit(
        shard_map(
            _body, mesh=mesh, in_specs=in_specs, out_specs=out_specs,
            check_rep=False,
        ),
        donate_argnums=donate,
        keep_unused=True,
    )
    return sharded, in_names, out_names, out_avals, nc


def _get_exec():
    global _EXEC
    if _EXEC is None:
        _EXEC = _build_exec()
    return _EXEC


def _selsum():
    s = np.zeros((P, GPC), np.float32)
    s[np.arange(P), np.arange(P) // GS] = 1.0
    return s


def _pair_fp8(w, boost):
    """[C_out, C_in] torch-style weight -> fp8 DR pair layout [NOP,P,2,C]."""
    wt = np.ascontiguousarray(np.asarray(w, np.float32).T) * boost
    return np.ascontiguousarray(
        wt.reshape(NOP, 2, P, C).transpose(0, 2, 1, 3)
    ).astype(FP8)


def make_concat_inputs(inputs):
    """Host-side prep: per-core shards concatenated on axis 0 (shard_map)."""
    x = np.asarray(inputs["x"], np.float32).reshape(B, C, N)
    sqs = np.float32(C ** -0.25)

    shared = {
        "wq": _pair_fp8(inputs["q_w"], sqs * QK_WBOOST),
        "wk": _pair_fp8(inputs["k_w"], sqs * QK_WBOOST),
        "wv": _pair_fp8(inputs["v_w"], V_WBOOST),
        "wp": _pair_fp8(inputs["proj_w"], P_WBOOST),
        # [gamma, beta, qb', kb', pb'] columns; pb' folds in proj_w @ v_b
        # (softmax weights sum to 1, so v_b is a constant proj-side shift)
        "vecs": np.ascontiguousarray(np.stack([
            np.asarray(inputs["gamma"], np.float32),
            np.asarray(inputs["beta"], np.float32),
            np.asarray(inputs["q_b"], np.float32)
            * (sqs * QK_WBOOST * QK_DSCALE),
            np.asarray(inputs["k_b"], np.float32)
            * (sqs * QK_WBOOST * QK_DSCALE),
            np.asarray(inputs["proj_b"], np.float32)
            + np.asarray(inputs["proj_w"], np.float32)
            @ np.asarray(inputs["v_b"], np.float32),
        ], axis=1)),
        "selsum": _selsum(),
        "selbc": np.ascontiguousarray(_selsum().T),
        "ones8": np.full((P, 2 * P), ONES_VAL, FP8),
    }
    per_core = [dict(shared, x=np.ascontiguousarray(x[c]),
                     xb=np.ascontiguousarray(x[c].astype(BF16)))
                for c in range(NCORES)]

    sharded, in_names, out_names, out_avals, _ = _get_exec()
    concat_in = [
        np.concatenate([per_core[c][nm] for c in range(NCORES)], axis=0)
        for nm in in_names
    ]
    return concat_in, out_avals


def run_concat(concat_in, out_avals):
    sharded = _get_exec()[0]
    concat_zeros = [
        np.zeros((NCORES * av.shape[0], *av.shape[1:]), av.dtype)
        for av in out_avals
    ]
    outs = sharded(*concat_in, *concat_zeros)
    return outs


def kernel(**inputs):
    concat_in, out_avals = make_concat_inputs(inputs)
    outs = run_concat(concat_in, out_avals)
    o = np.asarray(outs[0]).reshape(NCORES, C, N)
    return np.ascontiguousarray(o.reshape(B, C, H, W), dtype=np.float32)
